# revision 25
# baseline (speedup 1.0000x reference)
"""Trainium2 Bass kernel for BatchNorm2d + 8-head self-attention block.

Reference (per batch element b, all fp32):
    xn = BN_eval(x[b]); t = xn.T
    q/k/v = t @ W.T + b            # [S, 512], 8 heads x 64
    attn  = softmax(q k^T / 8)     # per head
    y[b]  = ((attn v) @ wo.T + bo).T

Sharding: pure data parallel — one batch element per NeuronCore, weights
replicated, no collectives.

Device design (per core), fully in the "transposed" domain (no large
transposes anywhere):
  - BN folded into QKV weights/biases on host; 1/8 scale folded into wq/bq;
    v bias folded into bo (softmax rows sum to 1).
  - Q^T,K^T [I,S] = wT.T @ x      (x arrives [C,S] — natural rhs)
  - V [S,I]       = x_chunk.T @ wvT, stored interleaved per head with a
    ones column ([128, 8*65]) so the PV matmul (M=65) also produces the
    softmax denominators for free.
  - scores^T per head [t,s]; head pairs row-packed via tile_position
    (0,0)/(64,0), K=64 each — the two row-group matmuls co-stream, so a
    scores pair costs one N=512 slot (~226ns). exp on ScalarE over both
    heads in one call (no max subtraction — scores are in [-3, 3]).
  - o^T accumulates over 8 t-chunks (K=128); normalize = approx-reciprocal
    row (read straight from PSUM) + partition-broadcast + DVE multiply;
    y^T = woT.T @ o^T + bo.

The wall-clock is paced by the ScalarE exp stream (64 ACTIVATEs x ~1.11us
on [128,1024]); the schedule keeps that stream dense:
  - ACT exp-table prefetched at t~0 with a dummy activation (saves the
    ~2.7us table-load off the first real exp).
  - DMA is ordered by need (x slab0, wq/wk head-pair 0, wv, x slab1,
    wq/wk rest, wo) so the first scores chunk fires ~5us in, not ~24us:
    wq/wk live in head-pair-major DRAM layout so the first 128 columns
    are one contiguous transfer.
  - All projection work is sprinkled as <=4-matmul fillers between
    attention chunks (PE has ~460ns slack per 1.11us exp), never at the
    first two chunks of a block, where the shared PSUM-bank pool is
    still held by the previous block's normalize.
  - Last block's normalize broadcasts 1/denom via a K=1 PE matmul into
    PSUM instead of the ~1us gpsimd partition_broadcast, shortening the
    tail after the final exp.

Matmul dtype is fp16: 2-byte weights keep LDWEIGHTS in the PE's
background buffer (hidden behind the previous matmul) where 4-byte fp32r
weights serialize ~150ns per matmul; fp16's 10-bit mantissa keeps the end
to-end error ~1e-3 of scale (all activations are within [-20, 20]).
"""

import numpy as np

import concourse.bass as bass
import concourse.tile as tile
from concourse import bacc, mybir
from concourse.bass_utils import run_bass_kernel_spmd
from concourse.tile import add_dep_helper

B, C, S = 8, 512, 1024
H, DH, INNER = 8, 64, 512
EPS = 1e-5
SCALE = DH ** (-0.5)
N_CORES = 8
F32 = mybir.dt.float32
F16 = mybir.dt.float16

DT_MM = F16

_CACHE: dict = {}

KC = C // 128      # 4 contraction chunks over channels
IT = INNER // 128  # 4 tiles over inner dim (also head-pair index)
ST = S // 128      # 8 t-chunks
NSLAB = S // 512   # 2 s-slabs


def build_bass(dt_mm):
    assert mybir.dt.size(dt_mm) == 2, "fp16/bf16 only"
    nc = bacc.Bacc("TRN2", target_bir_lowering=False, debug=False,
                   num_devices=N_CORES)

    # inputs arrive pre-arranged on the host into the SBUF layout
    # [partition, (head-pair,) k-chunk, free] so every DMA is contiguous
    # per partition AND the first head-pair of wq/wk is one transfer
    x_d = nc.dram_tensor("x", [128, KC, S], dt_mm, kind="ExternalInput")
    wqT_d = nc.dram_tensor("wqT", [128, IT, KC, 128], dt_mm,
                           kind="ExternalInput")
    wkT_d = nc.dram_tensor("wkT", [128, IT, KC, 128], dt_mm,
                           kind="ExternalInput")
    wvT_d = nc.dram_tensor("wvT", [128, KC, 512], dt_mm, kind="ExternalInput")
    woT_d = nc.dram_tensor("woT", [128, KC, 512], dt_mm, kind="ExternalInput")
    # bq | bk | bo packed on host as [128, 12] (col t+0/4/8 = vec[t*128+p])
    bias_d = nc.dram_tensor("bias_pack", [128, 3 * IT], F32, kind="ExternalInput")
    y_d = nc.dram_tensor("y", [C, S], F32, kind="ExternalOutput")

    with tile.TileContext(nc) as tc:
        with (
            tc.tile_pool(name="persist", bufs=1) as persist,
            tc.tile_pool(name="stage", bufs=2) as stage,
            tc.tile_pool(name="out", bufs=4) as outp,
            tc.tile_pool(name="norm", bufs=2) as normp,
            # one shared 4-slot pool for every 1-bank accumulator (projection
            # groups AND the two attention po accumulators): a fresh bank is
            # always available at head-pair transitions, so the strict-FIFO
            # PE queue never stalls behind the normalize chain.
            tc.tile_pool(name="psA", bufs=2, space="PSUM") as psA,
            tc.tile_pool(name="psP", bufs=2, space="PSUM") as psP,
            tc.tile_pool(name="psS", bufs=2, space="PSUM") as psS,
        ):
            xr = persist.tile([128, KC, S], dt_mm, tag="xr", name="xr")
            wqr = persist.tile([128, IT, KC, 128], dt_mm, tag="wqr", name="wqr")
            wkr = persist.tile([128, IT, KC, 128], dt_mm, tag="wkr", name="wkr")
            wvr = persist.tile([128, KC, 512], dt_mm, tag="wvr", name="wvr")
            wor = persist.tile([128, KC, 512], dt_mm, tag="wor", name="wor")

            # tiny bias pack rides the gpsimd/SWDGE queue in parallel with
            # the main chain; issue first so the Q7 starts immediately
            bias_sb = persist.tile([128, 3 * IT], F32, tag="bias")
            nc.gpsimd.dma_start(bias_sb[:], bias_d[:])
            bq_sb = bias_sb[:, 0:IT]
            bk_sb = bias_sb[:, IT:2 * IT]
            bo_sb = bias_sb[:, 2 * IT:3 * IT]

            # ---- main loads on BOTH HWDGE rings (sync + scalar), each a
            # need-ordered priority chain: a single ring streams ~140GB/s,
            # so x rides sync while the weights ride the scalar ring
            # concurrently (the scalar queue is idle until the first exp).
            dmas_sp = [
                nc.sync.dma_start(xr[:, :, 0:512], x_d[:, :, 0:512]),
                nc.sync.dma_start(xr[:, :, 512:1024], x_d[:, :, 512:1024]),
                nc.sync.dma_start(wor[:], woT_d[:]),
            ]
            dmas_act = [
                nc.scalar.dma_start(wqr[:, 0:1], wqT_d[:, 0:1]),
                nc.scalar.dma_start(wkr[:, 0:1], wkT_d[:, 0:1]),
                nc.scalar.dma_start(wvr[:], wvT_d[:]),
                nc.scalar.dma_start(wqr[:, 1:IT], wqT_d[:, 1:IT]),
                nc.scalar.dma_start(wkr[:, 1:IT], wkT_d[:, 1:IT]),
            ]
            for chain in (dmas_sp, dmas_act):
                for a, b in zip(chain[1:], chain):
                    add_dep_helper(a.ins, b.ins, sync=False,
                                   reason="dma priority")

            ones_sb = persist.tile([128, H], F32, tag="ones")
            nc.vector.memset(ones_sb[:], 1.0)
            # [1, 64] of ones: K=1 lhsT for the PE row-broadcast of 1/denom
            ones_row = persist.tile([1, 64], F32, tag="ones_row")
            nc.vector.memset(ones_row[:], 1.0)

            # ---- PE warmup + ACT table prefetch during the initial DMA
            # wait: ~2.6us of dummy matmuls keeps the HAM clock-gate fed
            # until real work arrives; the dummy exp pulls the ~2.7us
            # exp_and_others table load off the first real activation ----
            warm_sb = stage.tile([128, 256], dt_mm, tag="warm", bufs=1)
            nc.vector.memset(warm_sb[:], 0.0)
            dummy_act = stage.tile([1, 8], F32, tag="dummy", bufs=1)
            nc.scalar.activation(dummy_act[:], warm_sb[0:1, 0:8],
                                 mybir.ActivationFunctionType.Exp)
            warm_ps = psA.tile([128, 256], F32, tag="acc", name="warm_ps")
            NWARM = 22
            for wi in range(NWARM):
                nc.tensor.matmul(warm_ps[:], warm_sb[:, 0:128], warm_sb[:],
                                 start=(wi == 0), stop=(wi == NWARM - 1))

            # ---- persistent per-slab outputs ----
            qT = [[persist.tile([128, 512], dt_mm, tag=f"qT{i}{s}",
                                name=f"qT{i}{s}") for s in range(NSLAB)]
                  for i in range(IT)]
            kT = [[persist.tile([128, 512], dt_mm, tag=f"kT{i}{s}",
                                name=f"kT{i}{s}") for s in range(NSLAB)]
                  for i in range(IT)]
            oT = [[persist.tile([128, 512], dt_mm, tag=f"oT{i}{s}",
                                name=f"oT{i}{s}") for s in range(NSLAB)]
                  for i in range(IT)]
            v_sb = [persist.tile([128, H * 65], dt_mm, tag=f"v{t}",
                                 name=f"v{t}") for t in range(ST)]
            # 8 persistent exp tiles round-robin (instead of a pool): the
            # WAR on slot reuse then collapses into the same PE semaphore
            # the scores wait already uses, saving an EVENT_SEMAPHORE on
            # the ScalarE queue per chunk
            et_sb = [persist.tile([128, 1024], dt_mm, tag=f"et{j}",
                                  name=f"et{j}") for j in range(8)]
            et_ctr = [0]

            def group_thunks(n_mm, emit_mm, evac):
                """n_mm single-matmul thunks accumulating into one psA bank;
                the first allocates the bank, the last appends the evacuation.
                Each thunk returns the matmul instruction it emitted so the
                scheduler can pin it behind the current scores pair."""
                box = []

                def mk(i):
                    def t():
                        if i == 0:
                            box.append(psA.tile([128, 512], F32,
                                                tag="acc", name="acc"))
                        ins = emit_mm(box[0], i)
                        if i == n_mm - 1:
                            evac(box[0])
                        return ins
                    return t

                return [mk(i) for i in range(n_mm)]

            def qk_thunks(w, bias, dst, hp, sl, evac_eng="vector"):
                def emit_mm(ps, kc):
                    return nc.tensor.matmul(
                        ps[:],
                        w[:, hp, kc, :],
                        xr[:, kc, sl * 512:(sl + 1) * 512],
                        start=(kc == 0), stop=(kc == KC - 1),
                    )

                def evac(ps):
                    if evac_eng == "scalar":
                        # ScalarE is idle before the first exp; Copy is in
                        # every ACT table set and bias is a per-partition
                        # pointer, so this runs the bias-add off the DVE
                        nc.scalar.activation(
                            dst[hp][sl][:], ps[:],
                            mybir.ActivationFunctionType.Copy,
                            bias=bias[:, hp:hp + 1],
                        )
                    else:
                        nc.vector.tensor_scalar_add(
                            dst[hp][sl][:], ps[:], bias[:, hp:hp + 1]
                        )

                return group_thunks(KC, emit_mm, evac)

            def v_thunks(tc_):
                def emit_mm(ps, kc):
                    return nc.tensor.matmul(
                        ps[:],
                        xr[:, kc, tc_ * 128:(tc_ + 1) * 128],
                        wvr[:, kc, :],
                        start=(kc == 0), stop=(kc == KC - 1),
                    )

                def evac(ps):
                    vv = v_sb[tc_][:].rearrange("p (h m) -> p h m", h=H)
                    nc.vector.tensor_copy(
                        vv[:, :, 0:64], ps[:].rearrange("p (h m) -> p h m", h=H)
                    )
                    nc.vector.tensor_copy(vv[:, :, 64:65], ones_sb[:, :, None])

                return group_thunks(KC, emit_mm, evac)

            def run(thunks):
                for t in thunks:
                    t()

            class Blk:
                """One (slab, head-pair) attention block's emission pieces.

                Per-chunk emission order in the flat scheduler below:
                scores -> [PV pairs] -> fillers. PVs are BUNCHED into
                chunks 4-7 (two pairs per chunk, S+4PV = 1075ns < the
                1113ns exp), so this block's po PSUM banks are first
                written ~5.5us in — after the previous block's normalize
                has released its pair. The next block's scores(0) is
                emitted BETWEEN pv(6) and pv(7) so the exp stream crosses
                block boundaries with no PE work in front of it.
                """

                def __init__(self, sl, hp):
                    self.sl, self.hp = sl, hp
                    self.ets = []
                    self.po = None
                    self.rbc = None
                    self.last_mm = None

                def scores(self, tc_):
                    ksl, kcol = tc_ // 4, (tc_ % 4) * 128
                    pss = psS.tile([128, 1024], F32, tag="psS", name="psS")
                    nc.tensor.matmul(
                        pss[:, 0:512],
                        kT[self.hp][ksl][0:64, kcol:kcol + 128],
                        qT[self.hp][self.sl][0:64, :],
                        start=True, stop=True, tile_position=(0, 0),
                    )
                    self.last_mm = nc.tensor.matmul(
                        pss[:, 512:1024],
                        kT[self.hp][ksl][64:128, kcol:kcol + 128],
                        qT[self.hp][self.sl][64:128, :],
                        start=True, stop=True, tile_position=(64, 0),
                    )
                    et = et_sb[et_ctr[0] % 8]
                    et_ctr[0] += 1
                    nc.scalar.activation(
                        et[:], pss[:], mybir.ActivationFunctionType.Exp
                    )
                    self.ets.append(et)

                def pv(self, tc_):
                    if tc_ == 0:
                        self.po = (
                            psP.tile([65, 512], F32, tag="po", name="po0"),
                            psP.tile([65, 512], F32, tag="po", name="po1"),
                        )
                    for half in (0, 1):
                        h = 2 * self.hp + half
                        nc.tensor.matmul(
                            self.po[half][:],
                            v_sb[tc_][:, h * 65:(h + 1) * 65],
                            self.ets[tc_][:, half * 512:(half + 1) * 512],
                            start=(tc_ == 0), stop=(tc_ == ST - 1),
                        )

                def norm_front(self, last=False):
                    # stage the denominator rows to SBUF (DVE's iterative-
                    # divide op must not read PSUM directly; in the tail
                    # the idle ScalarE does the staging), then reciprocal +
                    # gpsimd partition-broadcast. The po*rbc muls are
                    # deferred into the NEXT block's emission (finish) so
                    # the DVE FIFO isn't head-blocked waiting on the
                    # broadcasts while later work is ready to run.
                    rrow = []
                    for half in (0, 1):
                        po = self.po[half]
                        dr = normp.tile([1, 512], F32, tag=f"drow{half}",
                                        name="drow")
                        if last:
                            nc.scalar.copy(dr[:], po[64:65, :])
                        else:
                            nc.vector.tensor_copy(dr[:], po[64:65, :])
                        rr = normp.tile([1, 512], F32, tag=f"rrow{half}",
                                        name="rrow")
                        nc.vector.reciprocal_approx_fast(rr[:], dr[:])
                        rrow.append(rr)
                    self.rbc = []
                    for half in (0, 1):
                        rb = normp.tile([64, 512], F32, tag=f"rbc{half}",
                                        name="rbc")
                        nc.gpsimd.partition_broadcast(rb[:], rrow[half][:])
                        self.rbc.append(rb)

                def finish(self):
                    for half in (0, 1):
                        nc.vector.tensor_mul(
                            oT[self.hp][self.sl][half * 64:(half + 1) * 64, :],
                            self.po[half][0:64, :],
                            self.rbc[half][:],
                        )

            def op_thunks(sl, ct):
                def emit_mm(ps, ic):
                    return nc.tensor.matmul(
                        ps[:],
                        wor[:, ic, ct * 128:(ct + 1) * 128],
                        oT[ic][sl][:],
                        start=(ic == 0), stop=(ic == IT - 1),
                    )

                def evac(ps):
                    ysb = outp.tile([128, 512], F32, tag="ysb", name="ysb")
                    nc.vector.tensor_scalar_add(ysb[:], ps[:],
                                                bo_sb[:, ct:ct + 1])
                    nc.sync.dma_start(
                        y_d[ct * 128:(ct + 1) * 128,
                            sl * 512:(sl + 1) * 512],
                        ysb[:],
                    )

                return group_thunks(IT, emit_mm, evac)

            y_part = [persist.tile([128, 512], F32, tag=f"yp{ct}",
                                   name=f"yp{ct}") for ct in range(IT)]

            def op_partial_thunks(ct):
                # ic 0..2 of the sl=1 projection, banked into SBUF (+bias)
                def emit_mm(ps, ic):
                    return nc.tensor.matmul(
                        ps[:],
                        wor[:, ic, ct * 128:(ct + 1) * 128],
                        oT[ic][1][:],
                        start=(ic == 0), stop=(ic == IT - 2),
                    )

                def evac(ps):
                    nc.vector.tensor_scalar_add(y_part[ct][:], ps[:],
                                                bo_sb[:, ct:ct + 1])

                return group_thunks(IT - 1, emit_mm, evac)

            y_fin = persist.tile([128, IT, 512], F32, tag="yfin",
                                 name="yfin")

            def op_final(ct):
                # psum for ct 0/1 borrows idle psS halves (scores are done)
                # so all four matmuls are in flight with no bank reuse wait
                if ct < 2:
                    pss = psS.tile([128, 1024], F32, tag="psS", name="psS")
                    ps = pss[:, 0:512]
                else:
                    ps = psA.tile([128, 512], F32, tag="acc", name="acc")[:]
                nc.tensor.matmul(
                    ps,
                    wor[:, IT - 1, ct * 128:(ct + 1) * 128],
                    oT[IT - 1][1][:],
                    start=True, stop=True,
                )
                nc.vector.tensor_add(y_fin[:, ct, :], y_part[ct][:], ps)

            def y_final_dma():
                # one 1MB store for the whole second output slab: one
                # completion receipt instead of four
                nc.sync.dma_start(
                    y_d[:, 512:1024].rearrange("(t p) s -> p t s", p=128),
                    y_fin[:],
                )

            # ---- emission order = static scheduler priority. Fillers are
            # single-matmul thunks so the exp-paced attention loop is never
            # blocked by a multi-matmul projection block sitting ahead of
            # the next scores in the PE's strict-FIFO queue.
            # Naming: Q(hp,sl)/K(hp,sl) 4-mm groups; V(t) 4-mm groups.
            # Deps: att(0,hp) reads q/k (hp,0) at chunk 0 and k (hp,1) at
            # chunk 4; v_sb[t] must land before chunk t's PV; oT[*][0] is
            # ready ~1.5us into block 4; oT[i][1] after block 4+i's
            # normalize. ----
            def Q(hp, sl):
                return qk_thunks(wqr, bq_sb, qT, hp, sl)

            def K(hp, sl):
                return qk_thunks(wkr, bk_sb, kT, hp, sl)

            # pre-phase (overlaps the DMA chains): first scores chunk needs
            # only Q(0,0)+K(0,0). Block 0 carries all eight V groups plus
            # the projections the next block reads; later blocks pop their
            # successors' q/k/op groups at chunks 0-2, by which point the
            # previous block's normalize has released the shared filler
            # PSUM banks. K(hp,1) is read by a block's own scores chunk 4,
            # so it pops at chunk 3 at the latest (or a block early).
            run(Q(0, 0))
            run(K(0, 0))
            # keep the PE's HAM clock-gate warm through the evac wait
            # before the first scores pair
            for wi in range(6):
                warm_ps2 = psA.tile([128, 256], F32, tag="acc",
                                    name="warm2") if wi == 0 else warm_ps2
                nc.tensor.matmul(warm_ps2[:], warm_sb[:, 0:128], warm_sb[:],
                                 start=(wi == 0), stop=(wi == 5))
            opc = [op_thunks(0, ct) for ct in range(IT)]
            op1p = op_partial_thunks(0) + op_partial_thunks(1) \
                + op_partial_thunks(2) + op_partial_thunks(3)
            fillers_tbl = [
                (v_thunks(0) + v_thunks(1) + v_thunks(2) + v_thunks(3)
                 + v_thunks(4) + v_thunks(5) + K(0, 1) + v_thunks(6)
                 + v_thunks(7) + Q(1, 0) + K(1, 0)),
                K(1, 1) + K(2, 1) + Q(2, 0) + K(2, 0),
                K(3, 1) + Q(3, 0) + K(3, 0),
                Q(0, 1) + Q(1, 1),
                Q(2, 1) + Q(3, 1),
                opc[0] + opc[1],
                opc[2] + opc[3],
                op1p,
            ]
            # pops start at chunk 1 so nothing sits between the boundary
            # scores pair and the next chunk's pair; blocks 6/7 carry the
            # output-projection work so the PE never runs so far ahead of
            # the exp stream that the psS reuse turns into a semaphore
            # ping-pong between the two engines.
            pops_tbl = [
                (8, 8, 8, 8, 8, 4, 0, 0),
                (0, 4, 4, 8, 0, 0, 0, 0),
                (0, 4, 4, 4, 0, 0, 0, 0),
                (0, 4, 4, 0, 0, 0, 0, 0),
                (0, 4, 4, 0, 0, 0, 0, 0),
                (0, 4, 4, 0, 0, 0, 0, 0),
                (0, 4, 4, 0, 0, 0, 0, 0),
                (0, 3, 3, 3, 3, 0, 0, 0),
            ]
            blocks = [Blk(sl, hp) for sl in range(NSLAB) for hp in range(IT)]

            def pop_filler(fillers, B):
                # every filler matmul gets a priority edge behind the most
                # recent scores pair: the Tile list-scheduler's DMA cost
                # model is optimistic, and without the edge it hoists
                # DMA-gated fillers ahead of ready scores in the PE FIFO,
                # stalling the exp stream behind the real (late) DMA.
                ins = fillers.pop(0)()
                if ins is not None and B.last_mm is not None:
                    add_dep_helper(ins.ins, B.last_mm.ins, sync=False,
                                   reason="filler after scores")

            for b, B in enumerate(blocks):
                fillers = list(fillers_tbl[b])
                pops = pops_tbl[b]
                pv_start = 4
                pv_done = 0
                for tc_ in range(ST):
                    if tc_ > 0 or b == 0:
                        B.scores(tc_)
                    if tc_ >= pv_start:
                        want = min(2 * (tc_ - pv_start + 1), ST)
                        while pv_done < want and pv_done <= tc_:
                            if pv_done == ST - 1 and b + 1 < len(blocks):
                                blocks[b + 1].scores(0)
                            B.pv(pv_done)
                            pv_done += 1
                    if tc_ == 1 and b > 0:
                        # before the pops: b7's op1p fillers read oT[2][1],
                        # which this finish writes — emission order is
                        # dependency order
                        blocks[b - 1].finish()
                    for _ in range(pops[tc_]):
                        if fillers:
                            pop_filler(fillers, B)
                B.norm_front(last=(b == len(blocks) - 1))
                while fillers:
                    pop_filler(fillers, B)
            # keep the PE warm through the final normalize chain so the
            # op_final matmuls run at full clock
            warm_ps3 = psA.tile([128, 256], F32, tag="acc", name="warm3")
            for wi in range(8):
                nc.tensor.matmul(warm_ps3[:], warm_sb[:, 0:128], warm_sb[:],
                                 start=(wi == 0), stop=(wi == 7))
            blocks[-1].finish()
            for ct in range(IT):
                op_final(ct)
            y_final_dma()

    nc.compile()
    return nc


def prep_host(inputs, dt_mm):
    """Fold BN + scale + v-bias into effective weights (fp32 numpy)."""
    x = np.asarray(inputs["x"], dtype=np.float32)
    g = np.asarray(inputs["bn_gamma"], dtype=np.float32)
    be = np.asarray(inputs["bn_beta"], dtype=np.float32)
    mu = np.asarray(inputs["bn_mean"], dtype=np.float32)
    var = np.asarray(inputs["bn_var"], dtype=np.float32)
    wq = np.asarray(inputs["wq"], dtype=np.float32)
    bq = np.asarray(inputs["bq"], dtype=np.float32)
    wk = np.asarray(inputs["wk"], dtype=np.float32)
    bk = np.asarray(inputs["bk"], dtype=np.float32)
    wv = np.asarray(inputs["wv"], dtype=np.float32)
    bv = np.asarray(inputs["bv"], dtype=np.float32)
    wo = np.asarray(inputs["wo"], dtype=np.float32)
    bo = np.asarray(inputs["bo"], dtype=np.float32)

    a = g / np.sqrt(var + EPS)          # [C]
    bvec = be - mu * a                  # [C]

    wq_eff = wq * a[None, :] * SCALE
    bq_eff = (bq + wq @ bvec) * SCALE
    wk_eff = wk * a[None, :]
    bk_eff = bk + wk @ bvec
    wv_eff = wv * a[None, :]
    bv_eff = bv + wv @ bvec
    bo_eff = bo + wo @ bv_eff           # v bias rides through softmax (sums to 1)

    bias_pack = np.concatenate(
        [bq_eff.reshape(IT, 128).T, bk_eff.reshape(IT, 128).T,
         bo_eff.reshape(IT, 128).T], axis=1
    ).astype(np.float32)

    np_dt = np.float16 if mybir.dt.size(dt_mm) == 2 else np.float32

    def dev_layout(a_):
        # [C_or_I, N] -> [128, KC, N]: partition p holds rows {k*128+p}
        return np.ascontiguousarray(
            a_.reshape(KC, 128, a_.shape[1]).transpose(1, 0, 2).astype(np_dt))

    def dev_layout_hp(a_):
        # [C, I] -> [128, IT, KC, 128]: head-pair-major so the first
        # head-pair's weights are one contiguous DMA
        return np.ascontiguousarray(
            a_.reshape(KC, 128, IT, 128).transpose(1, 2, 0, 3).astype(np_dt))

    wq_l = dev_layout_hp(wq_eff.T)
    wk_l = dev_layout_hp(wk_eff.T)
    wv_l = dev_layout(wv_eff.T)
    wo_l = dev_layout(wo.T)
    per_core = []
    for b in range(B):
        per_core.append({
            "x": dev_layout(x[b, :, :, 0]),
            "wqT": wq_l,
            "wkT": wk_l,
            "wvT": wv_l,
            "woT": wo_l,
            "bias_pack": np.ascontiguousarray(bias_pack),
        })
    return per_core


def _get_nc(dt_mm):
    key = str(dt_mm)
    if key not in _CACHE:
        _CACHE[key] = build_bass(dt_mm)
    return _CACHE[key]


def kernel(**inputs):
    nc = _get_nc(DT_MM)
    in_maps = prep_host(inputs, DT_MM)
    res = run_bass_kernel_spmd(nc, in_maps, list(range(N_CORES)))
    y = np.stack([res.results[c]["y"] for c in range(N_CORES)], axis=0)
    return y[..., None].astype(np.float32)


def run_traced(**inputs):
    """Like kernel() but with NTFF profiling; returns (y, results, tmpdir)."""
    nc = _get_nc(DT_MM)
    in_maps = prep_host(inputs, DT_MM)
    import tempfile
    tmpdir = tempfile.mkdtemp(prefix="mha_trace_")
    res = run_bass_kernel_spmd(
        nc, in_maps, list(range(N_CORES)), trace=True, tmpdir=tmpdir
    )
    y = np.stack([res.results[c]["y"] for c in range(N_CORES)], axis=0)
    return y[..., None].astype(np.float32), res, tmpdir


# revision 26
# speedup vs baseline: 1.0441x; 1.0441x over previous
"""Trainium2 Bass kernel for BatchNorm2d + 8-head self-attention block.

Reference (per batch element b, all fp32):
    xn = BN_eval(x[b]); t = xn.T
    q/k/v = t @ W.T + b            # [S, 512], 8 heads x 64
    attn  = softmax(q k^T / 8)     # per head
    y[b]  = ((attn v) @ wo.T + bo).T

Sharding: pure data parallel — one batch element per NeuronCore, weights
replicated, no collectives.

Device design (per core), fully in the "transposed" domain (no large
transposes anywhere):
  - BN folded into QKV weights/biases on host; 1/8 scale folded into wq/bq;
    v bias folded into bo (softmax rows sum to 1).
  - Q^T,K^T [I,S] = wT.T @ x      (x arrives [C,S] — natural rhs)
  - V [S,I]       = x_chunk.T @ wvT, stored interleaved per head with a
    ones column ([128, 8*65]) so the PV matmul (M=65) also produces the
    softmax denominators for free.
  - scores^T per head [t,s]; head pairs row-packed via tile_position
    (0,0)/(64,0), K=64 each — the two row-group matmuls co-stream, so a
    scores pair costs one N=512 slot (~226ns). exp on ScalarE over both
    heads in one call (no max subtraction — scores are in [-3, 3]).
  - o^T accumulates over 8 t-chunks (K=128); normalize = approx-reciprocal
    row (read straight from PSUM) + partition-broadcast + DVE multiply;
    y^T = woT.T @ o^T + bo.

The wall-clock is paced by the ScalarE exp stream (64 ACTIVATEs x ~1.11us
on [128,1024]); the schedule keeps that stream dense:
  - ACT exp-table prefetched at t~0 with a dummy activation (saves the
    ~2.7us table-load off the first real exp).
  - DMA is ordered by need (x slab0, wq/wk head-pair 0, wv, x slab1,
    wq/wk rest, wo) so the first scores chunk fires ~5us in, not ~24us:
    wq/wk live in head-pair-major DRAM layout so the first 128 columns
    are one contiguous transfer.
  - All projection work is sprinkled as <=4-matmul fillers between
    attention chunks (PE has ~460ns slack per 1.11us exp), never at the
    first two chunks of a block, where the shared PSUM-bank pool is
    still held by the previous block's normalize.
  - Last block's normalize broadcasts 1/denom via a K=1 PE matmul into
    PSUM instead of the ~1us gpsimd partition_broadcast, shortening the
    tail after the final exp.

Matmul dtype is fp16: 2-byte weights keep LDWEIGHTS in the PE's
background buffer (hidden behind the previous matmul) where 4-byte fp32r
weights serialize ~150ns per matmul; fp16's 10-bit mantissa keeps the end
to-end error ~1e-3 of scale (all activations are within [-20, 20]).
"""

import numpy as np

import concourse.bass as bass
import concourse.tile as tile
from concourse import bacc, mybir
from concourse.bass_utils import run_bass_kernel_spmd
from concourse.tile import add_dep_helper

B, C, S = 8, 512, 1024
H, DH, INNER = 8, 64, 512
EPS = 1e-5
SCALE = DH ** (-0.5)
N_CORES = 8
F32 = mybir.dt.float32
F16 = mybir.dt.float16

DT_MM = F16

_CACHE: dict = {}

KC = C // 128      # 4 contraction chunks over channels
IT = INNER // 128  # 4 tiles over inner dim (also head-pair index)
ST = S // 128      # 8 t-chunks
NSLAB = S // 512   # 2 s-slabs


def build_bass(dt_mm):
    assert mybir.dt.size(dt_mm) == 2, "fp16/bf16 only"
    nc = bacc.Bacc("TRN2", target_bir_lowering=False, debug=False,
                   num_devices=N_CORES)

    # inputs arrive pre-arranged on the host into the SBUF layout
    # [partition, (head-pair,) k-chunk, free] so every DMA is contiguous
    # per partition AND the first head-pair of wq/wk is one transfer
    x_d = nc.dram_tensor("x", [128, KC, S], dt_mm, kind="ExternalInput")
    wqT_d = nc.dram_tensor("wqT", [128, IT, KC, 128], dt_mm,
                           kind="ExternalInput")
    wkT_d = nc.dram_tensor("wkT", [128, IT, KC, 128], dt_mm,
                           kind="ExternalInput")
    wvT_d = nc.dram_tensor("wvT", [128, KC, 512], dt_mm, kind="ExternalInput")
    woT_d = nc.dram_tensor("woT", [128, KC, 512], dt_mm, kind="ExternalInput")
    # bq | bk | bo packed on host as [128, 12] (col t+0/4/8 = vec[t*128+p])
    bias_d = nc.dram_tensor("bias_pack", [128, 3 * IT], F32, kind="ExternalInput")
    y_d = nc.dram_tensor("y", [C, S], F32, kind="ExternalOutput")

    with tile.TileContext(nc) as tc:
        with (
            tc.tile_pool(name="persist", bufs=1) as persist,
            tc.tile_pool(name="stage", bufs=2) as stage,
            tc.tile_pool(name="out", bufs=4) as outp,
            tc.tile_pool(name="norm", bufs=2) as normp,
            # one shared 4-slot pool for every 1-bank accumulator (projection
            # groups AND the two attention po accumulators): a fresh bank is
            # always available at head-pair transitions, so the strict-FIFO
            # PE queue never stalls behind the normalize chain.
            tc.tile_pool(name="psA", bufs=2, space="PSUM") as psA,
            tc.tile_pool(name="psP", bufs=2, space="PSUM") as psP,
            tc.tile_pool(name="psS", bufs=2, space="PSUM") as psS,
        ):
            xr = persist.tile([128, KC, S], dt_mm, tag="xr", name="xr")
            wqr = persist.tile([128, IT, KC, 128], dt_mm, tag="wqr", name="wqr")
            wkr = persist.tile([128, IT, KC, 128], dt_mm, tag="wkr", name="wkr")
            wvr = persist.tile([128, KC, 512], dt_mm, tag="wvr", name="wvr")
            wor = persist.tile([128, KC, 512], dt_mm, tag="wor", name="wor")

            # tiny bias pack rides the gpsimd/SWDGE queue in parallel with
            # the main chain; issue first so the Q7 starts immediately
            bias_sb = persist.tile([128, 3 * IT], F32, tag="bias")
            nc.gpsimd.dma_start(bias_sb[:], bias_d[:])
            bq_sb = bias_sb[:, 0:IT]
            bk_sb = bias_sb[:, IT:2 * IT]
            bo_sb = bias_sb[:, 2 * IT:3 * IT]

            # ---- main loads on BOTH HWDGE rings (sync + scalar), each a
            # need-ordered priority chain: a single ring streams ~140GB/s,
            # so x rides sync while the weights ride the scalar ring
            # concurrently (the scalar queue is idle until the first exp).
            dmas_sp = [
                nc.sync.dma_start(xr[:, :, 0:512], x_d[:, :, 0:512]),
                nc.sync.dma_start(xr[:, :, 512:1024], x_d[:, :, 512:1024]),
                nc.sync.dma_start(wor[:], woT_d[:]),
            ]
            dmas_act = [
                nc.scalar.dma_start(wqr[:, 0:1], wqT_d[:, 0:1]),
                nc.scalar.dma_start(wkr[:, 0:1], wkT_d[:, 0:1]),
                nc.scalar.dma_start(wvr[:], wvT_d[:]),
                nc.scalar.dma_start(wqr[:, 1:IT], wqT_d[:, 1:IT]),
                nc.scalar.dma_start(wkr[:, 1:IT], wkT_d[:, 1:IT]),
            ]
            for chain in (dmas_sp, dmas_act):
                for a, b in zip(chain[1:], chain):
                    add_dep_helper(a.ins, b.ins, sync=False,
                                   reason="dma priority")

            ones_sb = persist.tile([128, H], F32, tag="ones")
            nc.vector.memset(ones_sb[:], 1.0)
            # [1, 64] of ones: K=1 lhsT for the PE row-broadcast of 1/denom
            ones_row = persist.tile([1, 64], F32, tag="ones_row")
            nc.vector.memset(ones_row[:], 1.0)

            # ---- PE warmup + ACT table prefetch during the initial DMA
            # wait: ~2.6us of dummy matmuls keeps the HAM clock-gate fed
            # until real work arrives; the dummy exp pulls the ~2.7us
            # exp_and_others table load off the first real activation ----
            warm_sb = stage.tile([128, 256], dt_mm, tag="warm", bufs=1)
            nc.vector.memset(warm_sb[:], 0.0)
            dummy_act = stage.tile([1, 8], F32, tag="dummy", bufs=1)
            nc.scalar.activation(dummy_act[:], warm_sb[0:1, 0:8],
                                 mybir.ActivationFunctionType.Exp)
            warm_ps = psA.tile([128, 256], F32, tag="acc", name="warm_ps")
            NWARM = 22
            for wi in range(NWARM):
                nc.tensor.matmul(warm_ps[:], warm_sb[:, 0:128], warm_sb[:],
                                 start=(wi == 0), stop=(wi == NWARM - 1))

            # ---- persistent per-slab outputs ----
            qT = [[persist.tile([128, 512], dt_mm, tag=f"qT{i}{s}",
                                name=f"qT{i}{s}") for s in range(NSLAB)]
                  for i in range(IT)]
            kT = [[persist.tile([128, 512], dt_mm, tag=f"kT{i}{s}",
                                name=f"kT{i}{s}") for s in range(NSLAB)]
                  for i in range(IT)]
            oT = [[persist.tile([128, 512], dt_mm, tag=f"oT{i}{s}",
                                name=f"oT{i}{s}") for s in range(NSLAB)]
                  for i in range(IT)]
            v_sb = [persist.tile([128, H * 65], dt_mm, tag=f"v{t}",
                                 name=f"v{t}") for t in range(ST)]
            # 8 persistent exp tiles round-robin (instead of a pool): the
            # WAR on slot reuse then collapses into the same PE semaphore
            # the scores wait already uses, saving an EVENT_SEMAPHORE on
            # the ScalarE queue per chunk
            et_sb = [persist.tile([128, 1024], dt_mm, tag=f"et{j}",
                                  name=f"et{j}") for j in range(8)]
            et_ctr = [0]

            def group_thunks(n_mm, emit_mm, evac):
                """n_mm single-matmul thunks accumulating into one psA bank;
                the first allocates the bank, the last appends the evacuation.
                Each thunk returns the matmul instruction it emitted so the
                scheduler can pin it behind the current scores pair."""
                box = []

                def mk(i):
                    def t():
                        if i == 0:
                            box.append(psA.tile([128, 512], F32,
                                                tag="acc", name="acc"))
                        ins = emit_mm(box[0], i)
                        if i == n_mm - 1:
                            evac(box[0])
                        return ins
                    return t

                return [mk(i) for i in range(n_mm)]

            def qk_thunks(w, bias, dst, hp, sl, evac_eng="vector"):
                def emit_mm(ps, kc):
                    return nc.tensor.matmul(
                        ps[:],
                        w[:, hp, kc, :],
                        xr[:, kc, sl * 512:(sl + 1) * 512],
                        start=(kc == 0), stop=(kc == KC - 1),
                    )

                def evac(ps):
                    if evac_eng == "scalar":
                        # ScalarE is idle before the first exp; Copy is in
                        # every ACT table set and bias is a per-partition
                        # pointer, so this runs the bias-add off the DVE
                        nc.scalar.activation(
                            dst[hp][sl][:], ps[:],
                            mybir.ActivationFunctionType.Copy,
                            bias=bias[:, hp:hp + 1],
                        )
                    else:
                        nc.vector.tensor_scalar_add(
                            dst[hp][sl][:], ps[:], bias[:, hp:hp + 1]
                        )

                return group_thunks(KC, emit_mm, evac)

            def v_thunks(tc_):
                def emit_mm(ps, kc):
                    return nc.tensor.matmul(
                        ps[:],
                        xr[:, kc, tc_ * 128:(tc_ + 1) * 128],
                        wvr[:, kc, :],
                        start=(kc == 0), stop=(kc == KC - 1),
                    )

                def evac(ps):
                    vv = v_sb[tc_][:].rearrange("p (h m) -> p h m", h=H)
                    nc.vector.tensor_copy(
                        vv[:, :, 0:64], ps[:].rearrange("p (h m) -> p h m", h=H)
                    )
                    nc.vector.tensor_copy(vv[:, :, 64:65], ones_sb[:, :, None])

                return group_thunks(KC, emit_mm, evac)

            def run(thunks):
                for t in thunks:
                    t()

            class Blk:
                """One (slab, head-pair) attention block's emission pieces.

                Per-chunk emission order in the flat scheduler below:
                scores -> [PV pairs] -> fillers. PVs are BUNCHED into
                chunks 4-7 (two pairs per chunk, S+4PV = 1075ns < the
                1113ns exp), so this block's po PSUM banks are first
                written ~5.5us in — after the previous block's normalize
                has released its pair. The next block's scores(0) is
                emitted BETWEEN pv(6) and pv(7) so the exp stream crosses
                block boundaries with no PE work in front of it.
                """

                def __init__(self, sl, hp):
                    self.sl, self.hp = sl, hp
                    self.ets = []
                    self.po = None
                    self.rbc = None
                    self.last_mm = None

                def scores(self, tc_):
                    ksl, kcol = tc_ // 4, (tc_ % 4) * 128
                    pss = psS.tile([128, 1024], F32, tag="psS", name="psS")
                    nc.tensor.matmul(
                        pss[:, 0:512],
                        kT[self.hp][ksl][0:64, kcol:kcol + 128],
                        qT[self.hp][self.sl][0:64, :],
                        start=True, stop=True, tile_position=(0, 0),
                    )
                    self.last_mm = nc.tensor.matmul(
                        pss[:, 512:1024],
                        kT[self.hp][ksl][64:128, kcol:kcol + 128],
                        qT[self.hp][self.sl][64:128, :],
                        start=True, stop=True, tile_position=(64, 0),
                    )
                    et = et_sb[et_ctr[0] % 8]
                    et_ctr[0] += 1
                    nc.scalar.activation(
                        et[:], pss[:], mybir.ActivationFunctionType.Exp
                    )
                    self.ets.append(et)

                def pv(self, tc_):
                    if tc_ == 0:
                        self.po = (
                            psP.tile([65, 512], F32, tag="po", name="po0"),
                            psP.tile([65, 512], F32, tag="po", name="po1"),
                        )
                    for half in (0, 1):
                        h = 2 * self.hp + half
                        nc.tensor.matmul(
                            self.po[half][:],
                            v_sb[tc_][:, h * 65:(h + 1) * 65],
                            self.ets[tc_][:, half * 512:(half + 1) * 512],
                            start=(tc_ == 0), stop=(tc_ == ST - 1),
                        )

                def norm_front(self, last=False):
                    # stage the denominator rows to SBUF (DVE's iterative-
                    # divide op must not read PSUM directly; in the tail
                    # the idle ScalarE does the staging), then reciprocal +
                    # gpsimd partition-broadcast. The po*rbc muls are
                    # deferred into the NEXT block's emission (finish) so
                    # the DVE FIFO isn't head-blocked waiting on the
                    # broadcasts while later work is ready to run.
                    rrow = []
                    for half in (0, 1):
                        po = self.po[half]
                        dr = normp.tile([1, 512], F32, tag=f"drow{half}",
                                        name="drow")
                        if last:
                            nc.scalar.copy(dr[:], po[64:65, :])
                        else:
                            nc.vector.tensor_copy(dr[:], po[64:65, :])
                        rr = normp.tile([1, 512], F32, tag=f"rrow{half}",
                                        name="rrow")
                        nc.vector.reciprocal_approx_fast(rr[:], dr[:])
                        rrow.append(rr)
                    self.rbc = []
                    for half in (0, 1):
                        rb = normp.tile([64, 512], F32, tag=f"rbc{half}",
                                        name="rbc")
                        nc.gpsimd.partition_broadcast(rb[:], rrow[half][:])
                        self.rbc.append(rb)

                def finish(self):
                    for half in (0, 1):
                        nc.vector.tensor_mul(
                            oT[self.hp][self.sl][half * 64:(half + 1) * 64, :],
                            self.po[half][0:64, :],
                            self.rbc[half][:],
                        )

            def op_thunks(sl, ct):
                def emit_mm(ps, ic):
                    return nc.tensor.matmul(
                        ps[:],
                        wor[:, ic, ct * 128:(ct + 1) * 128],
                        oT[ic][sl][:],
                        start=(ic == 0), stop=(ic == IT - 1),
                    )

                def evac(ps):
                    ysb = outp.tile([128, 512], F32, tag="ysb", name="ysb")
                    nc.vector.tensor_scalar_add(ysb[:], ps[:],
                                                bo_sb[:, ct:ct + 1])
                    nc.sync.dma_start(
                        y_d[ct * 128:(ct + 1) * 128,
                            sl * 512:(sl + 1) * 512],
                        ysb[:],
                    )

                return group_thunks(IT, emit_mm, evac)

            y_part = [persist.tile([128, 512], F32, tag=f"yp{ct}",
                                   name=f"yp{ct}") for ct in range(IT)]

            def op_partial_thunks(ct):
                # ic 0..2 of the sl=1 projection, banked into SBUF (+bias)
                def emit_mm(ps, ic):
                    return nc.tensor.matmul(
                        ps[:],
                        wor[:, ic, ct * 128:(ct + 1) * 128],
                        oT[ic][1][:],
                        start=(ic == 0), stop=(ic == IT - 2),
                    )

                def evac(ps):
                    nc.vector.tensor_scalar_add(y_part[ct][:], ps[:],
                                                bo_sb[:, ct:ct + 1])

                return group_thunks(IT - 1, emit_mm, evac)

            def op_final(ct):
                ps = psA.tile([128, 512], F32, tag="acc", name="acc")
                nc.tensor.matmul(
                    ps[:],
                    wor[:, IT - 1, ct * 128:(ct + 1) * 128],
                    oT[IT - 1][1][:],
                    start=True, stop=True,
                )
                ysb = outp.tile([128, 512], F32, tag="ysb", name="ysb")
                nc.vector.tensor_add(ysb[:], y_part[ct][:], ps[:])
                nc.sync.dma_start(
                    y_d[ct * 128:(ct + 1) * 128, 512:1024], ysb[:],
                )

            # ---- emission order = static scheduler priority. Fillers are
            # single-matmul thunks so the exp-paced attention loop is never
            # blocked by a multi-matmul projection block sitting ahead of
            # the next scores in the PE's strict-FIFO queue.
            # Naming: Q(hp,sl)/K(hp,sl) 4-mm groups; V(t) 4-mm groups.
            # Deps: att(0,hp) reads q/k (hp,0) at chunk 0 and k (hp,1) at
            # chunk 4; v_sb[t] must land before chunk t's PV; oT[*][0] is
            # ready ~1.5us into block 4; oT[i][1] after block 4+i's
            # normalize. ----
            def Q(hp, sl):
                return qk_thunks(wqr, bq_sb, qT, hp, sl)

            def K(hp, sl):
                return qk_thunks(wkr, bk_sb, kT, hp, sl)

            # pre-phase (overlaps the DMA chains): first scores chunk needs
            # only Q(0,0)+K(0,0). Block 0 carries all eight V groups plus
            # the projections the next block reads; later blocks pop their
            # successors' q/k/op groups at chunks 0-2, by which point the
            # previous block's normalize has released the shared filler
            # PSUM banks. K(hp,1) is read by a block's own scores chunk 4,
            # so it pops at chunk 3 at the latest (or a block early).
            run(Q(0, 0))
            run(K(0, 0))
            # keep the PE's HAM clock-gate warm through the evac wait
            # before the first scores pair
            for wi in range(6):
                warm_ps2 = psA.tile([128, 256], F32, tag="acc",
                                    name="warm2") if wi == 0 else warm_ps2
                nc.tensor.matmul(warm_ps2[:], warm_sb[:, 0:128], warm_sb[:],
                                 start=(wi == 0), stop=(wi == 5))
            opc = [op_thunks(0, ct) for ct in range(IT)]
            op1p = op_partial_thunks(0) + op_partial_thunks(1) \
                + op_partial_thunks(2) + op_partial_thunks(3)
            fillers_tbl = [
                (v_thunks(0) + v_thunks(1) + v_thunks(2) + v_thunks(3)
                 + v_thunks(4) + v_thunks(5) + K(0, 1) + v_thunks(6)
                 + v_thunks(7) + Q(1, 0) + K(1, 0)),
                K(1, 1) + K(2, 1) + Q(2, 0) + K(2, 0),
                K(3, 1) + Q(3, 0) + K(3, 0),
                Q(0, 1) + Q(1, 1),
                Q(2, 1) + Q(3, 1),
                opc[0] + opc[1],
                opc[2] + opc[3],
                op1p,
            ]
            # pops start at chunk 1 so nothing sits between the boundary
            # scores pair and the next chunk's pair; blocks 6/7 carry the
            # output-projection work so the PE never runs so far ahead of
            # the exp stream that the psS reuse turns into a semaphore
            # ping-pong between the two engines.
            pops_tbl = [
                (8, 8, 8, 8, 8, 4, 0, 0),
                (0, 4, 4, 8, 0, 0, 0, 0),
                (0, 4, 4, 4, 0, 0, 0, 0),
                (0, 4, 4, 0, 0, 0, 0, 0),
                (0, 4, 4, 0, 0, 0, 0, 0),
                (0, 4, 4, 0, 0, 0, 0, 0),
                (0, 4, 4, 0, 0, 0, 0, 0),
                (0, 3, 3, 3, 3, 0, 0, 0),
            ]
            blocks = [Blk(sl, hp) for sl in range(NSLAB) for hp in range(IT)]

            def pop_filler(fillers, B):
                # every filler matmul gets a priority edge behind the most
                # recent scores pair: the Tile list-scheduler's DMA cost
                # model is optimistic, and without the edge it hoists
                # DMA-gated fillers ahead of ready scores in the PE FIFO,
                # stalling the exp stream behind the real (late) DMA.
                ins = fillers.pop(0)()
                if ins is not None and B.last_mm is not None:
                    add_dep_helper(ins.ins, B.last_mm.ins, sync=False,
                                   reason="filler after scores")

            for b, B in enumerate(blocks):
                fillers = list(fillers_tbl[b])
                pops = pops_tbl[b]
                pv_start = 4
                pv_done = 0
                for tc_ in range(ST):
                    if tc_ > 0 or b == 0:
                        B.scores(tc_)
                    if tc_ >= pv_start:
                        want = min(2 * (tc_ - pv_start + 1), ST)
                        while pv_done < want and pv_done <= tc_:
                            if pv_done == ST - 1 and b + 1 < len(blocks):
                                blocks[b + 1].scores(0)
                            B.pv(pv_done)
                            pv_done += 1
                    if tc_ == 1 and b > 0:
                        # before the pops: b7's op1p fillers read oT[2][1],
                        # which this finish writes — emission order is
                        # dependency order
                        blocks[b - 1].finish()
                    for _ in range(pops[tc_]):
                        if fillers:
                            pop_filler(fillers, B)
                B.norm_front(last=(b == len(blocks) - 1))
                while fillers:
                    pop_filler(fillers, B)
            # keep the PE warm through the final normalize chain so the
            # op_final matmuls run at full clock
            warm_ps3 = psA.tile([128, 256], F32, tag="acc", name="warm3")
            for wi in range(8):
                nc.tensor.matmul(warm_ps3[:], warm_sb[:, 0:128], warm_sb[:],
                                 start=(wi == 0), stop=(wi == 7))
            blocks[-1].finish()
            for ct in range(IT):
                op_final(ct)

    nc.compile()
    return nc


def prep_host(inputs, dt_mm):
    """Fold BN + scale + v-bias into effective weights (fp32 numpy)."""
    x = np.asarray(inputs["x"], dtype=np.float32)
    g = np.asarray(inputs["bn_gamma"], dtype=np.float32)
    be = np.asarray(inputs["bn_beta"], dtype=np.float32)
    mu = np.asarray(inputs["bn_mean"], dtype=np.float32)
    var = np.asarray(inputs["bn_var"], dtype=np.float32)
    wq = np.asarray(inputs["wq"], dtype=np.float32)
    bq = np.asarray(inputs["bq"], dtype=np.float32)
    wk = np.asarray(inputs["wk"], dtype=np.float32)
    bk = np.asarray(inputs["bk"], dtype=np.float32)
    wv = np.asarray(inputs["wv"], dtype=np.float32)
    bv = np.asarray(inputs["bv"], dtype=np.float32)
    wo = np.asarray(inputs["wo"], dtype=np.float32)
    bo = np.asarray(inputs["bo"], dtype=np.float32)

    a = g / np.sqrt(var + EPS)          # [C]
    bvec = be - mu * a                  # [C]

    wq_eff = wq * a[None, :] * SCALE
    bq_eff = (bq + wq @ bvec) * SCALE
    wk_eff = wk * a[None, :]
    bk_eff = bk + wk @ bvec
    wv_eff = wv * a[None, :]
    bv_eff = bv + wv @ bvec
    bo_eff = bo + wo @ bv_eff           # v bias rides through softmax (sums to 1)

    bias_pack = np.concatenate(
        [bq_eff.reshape(IT, 128).T, bk_eff.reshape(IT, 128).T,
         bo_eff.reshape(IT, 128).T], axis=1
    ).astype(np.float32)

    np_dt = np.float16 if mybir.dt.size(dt_mm) == 2 else np.float32

    def dev_layout(a_):
        # [C_or_I, N] -> [128, KC, N]: partition p holds rows {k*128+p}
        return np.ascontiguousarray(
            a_.reshape(KC, 128, a_.shape[1]).transpose(1, 0, 2).astype(np_dt))

    def dev_layout_hp(a_):
        # [C, I] -> [128, IT, KC, 128]: head-pair-major so the first
        # head-pair's weights are one contiguous DMA
        return np.ascontiguousarray(
            a_.reshape(KC, 128, IT, 128).transpose(1, 2, 0, 3).astype(np_dt))

    wq_l = dev_layout_hp(wq_eff.T)
    wk_l = dev_layout_hp(wk_eff.T)
    wv_l = dev_layout(wv_eff.T)
    wo_l = dev_layout(wo.T)
    per_core = []
    for b in range(B):
        per_core.append({
            "x": dev_layout(x[b, :, :, 0]),
            "wqT": wq_l,
            "wkT": wk_l,
            "wvT": wv_l,
            "woT": wo_l,
            "bias_pack": np.ascontiguousarray(bias_pack),
        })
    return per_core


def _get_nc(dt_mm):
    key = str(dt_mm)
    if key not in _CACHE:
        _CACHE[key] = build_bass(dt_mm)
    return _CACHE[key]


def kernel(**inputs):
    nc = _get_nc(DT_MM)
    in_maps = prep_host(inputs, DT_MM)
    res = run_bass_kernel_spmd(nc, in_maps, list(range(N_CORES)))
    y = np.stack([res.results[c]["y"] for c in range(N_CORES)], axis=0)
    return y[..., None].astype(np.float32)


def run_traced(**inputs):
    """Like kernel() but with NTFF profiling; returns (y, results, tmpdir)."""
    nc = _get_nc(DT_MM)
    in_maps = prep_host(inputs, DT_MM)
    import tempfile
    tmpdir = tempfile.mkdtemp(prefix="mha_trace_")
    res = run_bass_kernel_spmd(
        nc, in_maps, list(range(N_CORES)), trace=True, tmpdir=tmpdir
    )
    y = np.stack([res.results[c]["y"] for c in range(N_CORES)], axis=0)
    return y[..., None].astype(np.float32), res, tmpdir


# revision 27
# speedup vs baseline: 1.0529x; 1.0085x over previous
"""Trainium2 Bass kernel for BatchNorm2d + 8-head self-attention block.

Reference (per batch element b, all fp32):
    xn = BN_eval(x[b]); t = xn.T
    q/k/v = t @ W.T + b            # [S, 512], 8 heads x 64
    attn  = softmax(q k^T / 8)     # per head
    y[b]  = ((attn v) @ wo.T + bo).T

Sharding: pure data parallel — one batch element per NeuronCore, weights
replicated, no collectives.

Device design (per core), fully in the "transposed" domain (no large
transposes anywhere):
  - BN folded into QKV weights/biases on host; 1/8 scale folded into wq/bq;
    v bias folded into bo (softmax rows sum to 1).
  - Q^T,K^T [I,S] = wT.T @ x      (x arrives [C,S] — natural rhs)
  - V [S,I]       = x_chunk.T @ wvT, stored interleaved per head with a
    ones column ([128, 8*65]) so the PV matmul (M=65) also produces the
    softmax denominators for free.
  - scores^T per head [t,s]; head pairs row-packed via tile_position
    (0,0)/(64,0), K=64 each — the two row-group matmuls co-stream, so a
    scores pair costs one N=512 slot (~226ns). exp on ScalarE over both
    heads in one call (no max subtraction — scores are in [-3, 3]).
  - o^T accumulates over 8 t-chunks (K=128); normalize = stage the
    denominator row to SBUF (DVE's iterative-divide op must not read
    PSUM directly — doing so NaNs on hardware), approx-reciprocal,
    gpsimd partition-broadcast, DVE multiply; y^T = woT.T @ o^T + bo.

The wall-clock is paced by the ScalarE exp stream (64 ACTIVATEs x ~1.11us
on [128,1024]); the schedule keeps that stream dense:
  - ACT exp-table prefetched at t~0 with a dummy activation (saves the
    ~2.7us table-load off the first real exp); ~4.7us of dummy matmuls
    keep the PE's HAM clock-gate at 8/8 through the initial DMA wait.
  - inputs ride BOTH HWDGE rings concurrently (x + wo on sync, weights
    on the then-idle scalar ring), each ring a need-ordered priority
    chain; wq/wk live in head-pair-major DRAM layout so head-pair 0 is
    one contiguous early transfer. First exp fires ~15us in, not ~24us.
  - flat 64-chunk emission: scores pairs always lead; PV pairs are
    bunched into chunks 4-7 of each block (after the previous block's
    normalize has released the two po PSUM banks), and the next block's
    scores(0) is emitted between pv(6) and pv(7) so the exp stream
    crosses block boundaries with nothing in front of it. Projection /
    output fillers pop at chunks 1-3 from their own 2-bank PSUM pool,
    each pinned behind the current scores pair with a priority edge
    (the list-scheduler's optimistic DMA model would otherwise hoist
    DMA-gated fillers ahead of ready scores). The po*rbc normalize muls
    are deferred into the next block so the DVE FIFO is never
    head-blocked on the gpsimd broadcasts.
  - the denominator staging of the last block runs on the then-idle
    ScalarE, and dummy matmuls keep the PE warm through the final
    normalize so the last output projections run at full clock.

Matmul dtype is fp16: 2-byte weights keep LDWEIGHTS in the PE's
background buffer (hidden behind the previous matmul) where 4-byte fp32r
weights serialize ~150ns per matmul; fp16's 10-bit mantissa keeps the end
to-end error ~1e-3 of scale (all activations are within [-20, 20]).
"""

import numpy as np

import concourse.bass as bass
import concourse.tile as tile
from concourse import bacc, mybir
from concourse.bass_utils import run_bass_kernel_spmd
from concourse.tile import add_dep_helper

B, C, S = 8, 512, 1024
H, DH, INNER = 8, 64, 512
EPS = 1e-5
SCALE = DH ** (-0.5)
N_CORES = 8
F32 = mybir.dt.float32
F16 = mybir.dt.float16

DT_MM = F16

_CACHE: dict = {}

KC = C // 128      # 4 contraction chunks over channels
IT = INNER // 128  # 4 tiles over inner dim (also head-pair index)
ST = S // 128      # 8 t-chunks
NSLAB = S // 512   # 2 s-slabs


def build_bass(dt_mm):
    assert mybir.dt.size(dt_mm) == 2, "fp16/bf16 only"
    nc = bacc.Bacc("TRN2", target_bir_lowering=False, debug=False,
                   num_devices=N_CORES)

    # inputs arrive pre-arranged on the host into the SBUF layout
    # [partition, (head-pair,) k-chunk, free] so every DMA is contiguous
    # per partition AND the first head-pair of wq/wk is one transfer
    x_d = nc.dram_tensor("x", [128, KC, S], dt_mm, kind="ExternalInput")
    wqT_d = nc.dram_tensor("wqT", [128, IT, KC, 128], dt_mm,
                           kind="ExternalInput")
    wkT_d = nc.dram_tensor("wkT", [128, IT, KC, 128], dt_mm,
                           kind="ExternalInput")
    wvT_d = nc.dram_tensor("wvT", [128, KC, 512], dt_mm, kind="ExternalInput")
    woT_d = nc.dram_tensor("woT", [128, KC, 512], dt_mm, kind="ExternalInput")
    # bq | bk | bo packed on host as [128, 12] (col t+0/4/8 = vec[t*128+p])
    bias_d = nc.dram_tensor("bias_pack", [128, 3 * IT], F32, kind="ExternalInput")
    y_d = nc.dram_tensor("y", [C, S], F32, kind="ExternalOutput")

    with tile.TileContext(nc) as tc:
        with (
            tc.tile_pool(name="persist", bufs=1) as persist,
            tc.tile_pool(name="stage", bufs=2) as stage,
            tc.tile_pool(name="out", bufs=4) as outp,
            tc.tile_pool(name="norm", bufs=2) as normp,
            # one shared 4-slot pool for every 1-bank accumulator (projection
            # groups AND the two attention po accumulators): a fresh bank is
            # always available at head-pair transitions, so the strict-FIFO
            # PE queue never stalls behind the normalize chain.
            tc.tile_pool(name="psA", bufs=2, space="PSUM") as psA,
            tc.tile_pool(name="psP", bufs=2, space="PSUM") as psP,
            tc.tile_pool(name="psS", bufs=2, space="PSUM") as psS,
        ):
            xr = persist.tile([128, KC, S], dt_mm, tag="xr", name="xr")
            wqr = persist.tile([128, IT, KC, 128], dt_mm, tag="wqr", name="wqr")
            wkr = persist.tile([128, IT, KC, 128], dt_mm, tag="wkr", name="wkr")
            wvr = persist.tile([128, KC, 512], dt_mm, tag="wvr", name="wvr")
            wor = persist.tile([128, KC, 512], dt_mm, tag="wor", name="wor")

            # tiny bias pack rides the gpsimd/SWDGE queue in parallel with
            # the main chain; issue first so the Q7 starts immediately
            bias_sb = persist.tile([128, 3 * IT], F32, tag="bias")
            nc.gpsimd.dma_start(bias_sb[:], bias_d[:])
            bq_sb = bias_sb[:, 0:IT]
            bk_sb = bias_sb[:, IT:2 * IT]
            bo_sb = bias_sb[:, 2 * IT:3 * IT]

            # ---- main loads on BOTH HWDGE rings (sync + scalar), each a
            # need-ordered priority chain: a single ring streams ~140GB/s,
            # so x rides sync while the weights ride the scalar ring
            # concurrently (the scalar queue is idle until the first exp).
            dmas_sp = [
                nc.sync.dma_start(xr[:, :, 0:512], x_d[:, :, 0:512]),
                nc.sync.dma_start(xr[:, :, 512:1024], x_d[:, :, 512:1024]),
                nc.sync.dma_start(wor[:], woT_d[:]),
            ]
            dmas_act = [
                nc.scalar.dma_start(wqr[:, 0:1], wqT_d[:, 0:1]),
                nc.scalar.dma_start(wkr[:, 0:1], wkT_d[:, 0:1]),
                nc.scalar.dma_start(wvr[:], wvT_d[:]),
                nc.scalar.dma_start(wqr[:, 1:IT], wqT_d[:, 1:IT]),
                nc.scalar.dma_start(wkr[:, 1:IT], wkT_d[:, 1:IT]),
            ]
            for chain in (dmas_sp, dmas_act):
                for a, b in zip(chain[1:], chain):
                    add_dep_helper(a.ins, b.ins, sync=False,
                                   reason="dma priority")

            ones_sb = persist.tile([128, H], F32, tag="ones")
            nc.vector.memset(ones_sb[:], 1.0)

            # ---- PE warmup + ACT table prefetch during the initial DMA
            # wait: ~2.6us of dummy matmuls keeps the HAM clock-gate fed
            # until real work arrives; the dummy exp pulls the ~2.7us
            # exp_and_others table load off the first real activation ----
            warm_sb = stage.tile([128, 256], dt_mm, tag="warm", bufs=1)
            nc.vector.memset(warm_sb[:], 0.0)
            dummy_act = stage.tile([1, 8], F32, tag="dummy", bufs=1)
            nc.scalar.activation(dummy_act[:], warm_sb[0:1, 0:8],
                                 mybir.ActivationFunctionType.Exp)
            warm_ps = psA.tile([128, 256], F32, tag="acc", name="warm_ps")
            NWARM = 22
            for wi in range(NWARM):
                nc.tensor.matmul(warm_ps[:], warm_sb[:, 0:128], warm_sb[:],
                                 start=(wi == 0), stop=(wi == NWARM - 1))

            # ---- persistent per-slab outputs ----
            qT = [[persist.tile([128, 512], dt_mm, tag=f"qT{i}{s}",
                                name=f"qT{i}{s}") for s in range(NSLAB)]
                  for i in range(IT)]
            kT = [[persist.tile([128, 512], dt_mm, tag=f"kT{i}{s}",
                                name=f"kT{i}{s}") for s in range(NSLAB)]
                  for i in range(IT)]
            oT = [[persist.tile([128, 512], dt_mm, tag=f"oT{i}{s}",
                                name=f"oT{i}{s}") for s in range(NSLAB)]
                  for i in range(IT)]
            v_sb = [persist.tile([128, H * 65], dt_mm, tag=f"v{t}",
                                 name=f"v{t}") for t in range(ST)]
            # 8 persistent exp tiles round-robin (instead of a pool): the
            # WAR on slot reuse then collapses into the same PE semaphore
            # the scores wait already uses, saving an EVENT_SEMAPHORE on
            # the ScalarE queue per chunk
            et_sb = [persist.tile([128, 1024], dt_mm, tag=f"et{j}",
                                  name=f"et{j}") for j in range(8)]
            et_ctr = [0]

            def group_thunks(n_mm, emit_mm, evac):
                """n_mm single-matmul thunks accumulating into one psA bank;
                the first allocates the bank, the last appends the evacuation.
                Each thunk returns the matmul instruction it emitted so the
                scheduler can pin it behind the current scores pair."""
                box = []

                def mk(i):
                    def t():
                        if i == 0:
                            box.append(psA.tile([128, 512], F32,
                                                tag="acc", name="acc"))
                        ins = emit_mm(box[0], i)
                        if i == n_mm - 1:
                            evac(box[0])
                        return ins
                    return t

                return [mk(i) for i in range(n_mm)]

            def qk_thunks(w, bias, dst, hp, sl, evac_eng="vector"):
                def emit_mm(ps, kc):
                    return nc.tensor.matmul(
                        ps[:],
                        w[:, hp, kc, :],
                        xr[:, kc, sl * 512:(sl + 1) * 512],
                        start=(kc == 0), stop=(kc == KC - 1),
                    )

                def evac(ps):
                    if evac_eng == "scalar":
                        # ScalarE is idle before the first exp; Copy is in
                        # every ACT table set and bias is a per-partition
                        # pointer, so this runs the bias-add off the DVE
                        nc.scalar.activation(
                            dst[hp][sl][:], ps[:],
                            mybir.ActivationFunctionType.Copy,
                            bias=bias[:, hp:hp + 1],
                        )
                    else:
                        nc.vector.tensor_scalar_add(
                            dst[hp][sl][:], ps[:], bias[:, hp:hp + 1]
                        )

                return group_thunks(KC, emit_mm, evac)

            def v_thunks(tc_):
                def emit_mm(ps, kc):
                    return nc.tensor.matmul(
                        ps[:],
                        xr[:, kc, tc_ * 128:(tc_ + 1) * 128],
                        wvr[:, kc, :],
                        start=(kc == 0), stop=(kc == KC - 1),
                    )

                def evac(ps):
                    vv = v_sb[tc_][:].rearrange("p (h m) -> p h m", h=H)
                    nc.vector.tensor_copy(
                        vv[:, :, 0:64], ps[:].rearrange("p (h m) -> p h m", h=H)
                    )
                    nc.vector.tensor_copy(vv[:, :, 64:65], ones_sb[:, :, None])

                return group_thunks(KC, emit_mm, evac)

            def run(thunks):
                for t in thunks:
                    t()

            class Blk:
                """One (slab, head-pair) attention block's emission pieces.

                Per-chunk emission order in the flat scheduler below:
                scores -> [PV pairs] -> fillers. PVs are BUNCHED into
                chunks 4-7 (two pairs per chunk, S+4PV = 1075ns < the
                1113ns exp), so this block's po PSUM banks are first
                written ~5.5us in — after the previous block's normalize
                has released its pair. The next block's scores(0) is
                emitted BETWEEN pv(6) and pv(7) so the exp stream crosses
                block boundaries with no PE work in front of it.
                """

                def __init__(self, sl, hp):
                    self.sl, self.hp = sl, hp
                    self.ets = []
                    self.po = None
                    self.rbc = None
                    self.last_mm = None

                def scores(self, tc_):
                    ksl, kcol = tc_ // 4, (tc_ % 4) * 128
                    pss = psS.tile([128, 1024], F32, tag="psS", name="psS")
                    nc.tensor.matmul(
                        pss[:, 0:512],
                        kT[self.hp][ksl][0:64, kcol:kcol + 128],
                        qT[self.hp][self.sl][0:64, :],
                        start=True, stop=True, tile_position=(0, 0),
                    )
                    self.last_mm = nc.tensor.matmul(
                        pss[:, 512:1024],
                        kT[self.hp][ksl][64:128, kcol:kcol + 128],
                        qT[self.hp][self.sl][64:128, :],
                        start=True, stop=True, tile_position=(64, 0),
                    )
                    et = et_sb[et_ctr[0] % 8]
                    et_ctr[0] += 1
                    nc.scalar.activation(
                        et[:], pss[:], mybir.ActivationFunctionType.Exp
                    )
                    self.ets.append(et)

                def pv(self, tc_):
                    if tc_ == 0:
                        self.po = (
                            psP.tile([65, 512], F32, tag="po", name="po0"),
                            psP.tile([65, 512], F32, tag="po", name="po1"),
                        )
                    for half in (0, 1):
                        h = 2 * self.hp + half
                        nc.tensor.matmul(
                            self.po[half][:],
                            v_sb[tc_][:, h * 65:(h + 1) * 65],
                            self.ets[tc_][:, half * 512:(half + 1) * 512],
                            start=(tc_ == 0), stop=(tc_ == ST - 1),
                        )

                def norm_front(self, last=False):
                    # stage the denominator rows to SBUF (DVE's iterative-
                    # divide op must not read PSUM directly; in the tail
                    # the idle ScalarE does the staging), then reciprocal +
                    # gpsimd partition-broadcast. The po*rbc muls are
                    # deferred into the NEXT block's emission (finish) so
                    # the DVE FIFO isn't head-blocked waiting on the
                    # broadcasts while later work is ready to run.
                    rrow = []
                    for half in (0, 1):
                        po = self.po[half]
                        dr = normp.tile([1, 512], F32, tag=f"drow{half}",
                                        name="drow")
                        if last:
                            nc.scalar.copy(dr[:], po[64:65, :])
                        else:
                            nc.vector.tensor_copy(dr[:], po[64:65, :])
                        rr = normp.tile([1, 512], F32, tag=f"rrow{half}",
                                        name="rrow")
                        nc.vector.reciprocal_approx_fast(rr[:], dr[:])
                        rrow.append(rr)
                    self.rbc = []
                    for half in (0, 1):
                        rb = normp.tile([64, 512], F32, tag=f"rbc{half}",
                                        name="rbc")
                        nc.gpsimd.partition_broadcast(rb[:], rrow[half][:])
                        self.rbc.append(rb)

                def finish(self):
                    for half in (0, 1):
                        nc.vector.tensor_mul(
                            oT[self.hp][self.sl][half * 64:(half + 1) * 64, :],
                            self.po[half][0:64, :],
                            self.rbc[half][:],
                        )

            def op_thunks(sl, ct):
                def emit_mm(ps, ic):
                    return nc.tensor.matmul(
                        ps[:],
                        wor[:, ic, ct * 128:(ct + 1) * 128],
                        oT[ic][sl][:],
                        start=(ic == 0), stop=(ic == IT - 1),
                    )

                def evac(ps):
                    ysb = outp.tile([128, 512], F32, tag="ysb", name="ysb")
                    nc.vector.tensor_scalar_add(ysb[:], ps[:],
                                                bo_sb[:, ct:ct + 1])
                    nc.sync.dma_start(
                        y_d[ct * 128:(ct + 1) * 128,
                            sl * 512:(sl + 1) * 512],
                        ysb[:],
                    )

                return group_thunks(IT, emit_mm, evac)

            y_part = [persist.tile([128, 512], F32, tag=f"yp{ct}",
                                   name=f"yp{ct}") for ct in range(IT)]

            def op_partial_thunks(ct):
                # ic 0..2 of the sl=1 projection, banked into SBUF (+bias)
                def emit_mm(ps, ic):
                    return nc.tensor.matmul(
                        ps[:],
                        wor[:, ic, ct * 128:(ct + 1) * 128],
                        oT[ic][1][:],
                        start=(ic == 0), stop=(ic == IT - 2),
                    )

                def evac(ps):
                    nc.vector.tensor_scalar_add(y_part[ct][:], ps[:],
                                                bo_sb[:, ct:ct + 1])

                return group_thunks(IT - 1, emit_mm, evac)

            def op_final(ct):
                ps = psA.tile([128, 512], F32, tag="acc", name="acc")
                nc.tensor.matmul(
                    ps[:],
                    wor[:, IT - 1, ct * 128:(ct + 1) * 128],
                    oT[IT - 1][1][:],
                    start=True, stop=True,
                )
                ysb = outp.tile([128, 512], F32, tag="ysb", name="ysb")
                nc.vector.tensor_add(ysb[:], y_part[ct][:], ps[:])
                nc.sync.dma_start(
                    y_d[ct * 128:(ct + 1) * 128, 512:1024], ysb[:],
                )

            # ---- emission order = static scheduler priority. Fillers are
            # single-matmul thunks so the exp-paced attention loop is never
            # blocked by a multi-matmul projection block sitting ahead of
            # the next scores in the PE's strict-FIFO queue.
            # Naming: Q(hp,sl)/K(hp,sl) 4-mm groups; V(t) 4-mm groups.
            # Deps: att(0,hp) reads q/k (hp,0) at chunk 0 and k (hp,1) at
            # chunk 4; v_sb[t] must land before chunk t's PV; oT[*][0] is
            # ready ~1.5us into block 4; oT[i][1] after block 4+i's
            # normalize. ----
            def Q(hp, sl):
                return qk_thunks(wqr, bq_sb, qT, hp, sl)

            def K(hp, sl):
                return qk_thunks(wkr, bk_sb, kT, hp, sl)

            # pre-phase (overlaps the DMA chains): first scores chunk needs
            # only Q(0,0)+K(0,0). Block 0 carries all eight V groups plus
            # the projections the next block reads; later blocks pop their
            # successors' q/k/op groups at chunks 0-2, by which point the
            # previous block's normalize has released the shared filler
            # PSUM banks. K(hp,1) is read by a block's own scores chunk 4,
            # so it pops at chunk 3 at the latest (or a block early).
            run(Q(0, 0))
            run(K(0, 0))
            # keep the PE's HAM clock-gate warm through the evac wait
            # before the first scores pair
            for wi in range(6):
                warm_ps2 = psA.tile([128, 256], F32, tag="acc",
                                    name="warm2") if wi == 0 else warm_ps2
                nc.tensor.matmul(warm_ps2[:], warm_sb[:, 0:128], warm_sb[:],
                                 start=(wi == 0), stop=(wi == 5))
            opc = [op_thunks(0, ct) for ct in range(IT)]
            op1p = op_partial_thunks(0) + op_partial_thunks(1) \
                + op_partial_thunks(2) + op_partial_thunks(3)
            fillers_tbl = [
                (v_thunks(0) + v_thunks(1) + v_thunks(2) + v_thunks(3)
                 + v_thunks(4) + v_thunks(5) + K(0, 1) + v_thunks(6)
                 + v_thunks(7) + Q(1, 0) + K(1, 0)),
                K(1, 1) + K(2, 1) + Q(2, 0) + K(2, 0),
                K(3, 1) + Q(3, 0) + K(3, 0),
                Q(0, 1) + Q(1, 1),
                Q(2, 1) + Q(3, 1),
                opc[0] + opc[1],
                opc[2] + opc[3],
                op1p,
            ]
            # pops start at chunk 1 so nothing sits between the boundary
            # scores pair and the next chunk's pair; blocks 6/7 carry the
            # output-projection work so the PE never runs so far ahead of
            # the exp stream that the psS reuse turns into a semaphore
            # ping-pong between the two engines.
            pops_tbl = [
                (8, 8, 8, 8, 8, 4, 0, 0),
                (0, 4, 4, 8, 0, 0, 0, 0),
                (0, 4, 4, 4, 0, 0, 0, 0),
                (0, 4, 4, 0, 0, 0, 0, 0),
                (0, 4, 4, 0, 0, 0, 0, 0),
                (0, 4, 4, 0, 0, 0, 0, 0),
                (0, 4, 4, 0, 0, 0, 0, 0),
                (0, 3, 3, 3, 3, 0, 0, 0),
            ]
            blocks = [Blk(sl, hp) for sl in range(NSLAB) for hp in range(IT)]

            def pop_filler(fillers, B):
                # every filler matmul gets a priority edge behind the most
                # recent scores pair: the Tile list-scheduler's DMA cost
                # model is optimistic, and without the edge it hoists
                # DMA-gated fillers ahead of ready scores in the PE FIFO,
                # stalling the exp stream behind the real (late) DMA.
                ins = fillers.pop(0)()
                if ins is not None and B.last_mm is not None:
                    add_dep_helper(ins.ins, B.last_mm.ins, sync=False,
                                   reason="filler after scores")

            for b, B in enumerate(blocks):
                fillers = list(fillers_tbl[b])
                pops = pops_tbl[b]
                pv_start = 4
                pv_done = 0
                for tc_ in range(ST):
                    if tc_ > 0 or b == 0:
                        B.scores(tc_)
                    if tc_ >= pv_start:
                        want = min(2 * (tc_ - pv_start + 1), ST)
                        while pv_done < want and pv_done <= tc_:
                            if pv_done == ST - 1 and b + 1 < len(blocks):
                                blocks[b + 1].scores(0)
                            B.pv(pv_done)
                            pv_done += 1
                    if tc_ == 1 and b > 0:
                        # before the pops: b7's op1p fillers read oT[2][1],
                        # which this finish writes — emission order is
                        # dependency order
                        blocks[b - 1].finish()
                    for _ in range(pops[tc_]):
                        if fillers:
                            pop_filler(fillers, B)
                B.norm_front(last=(b == len(blocks) - 1))
                while fillers:
                    pop_filler(fillers, B)
            # keep the PE warm through the final normalize chain so the
            # op_final matmuls run at full clock
            warm_ps3 = psA.tile([128, 256], F32, tag="acc", name="warm3")
            for wi in range(8):
                nc.tensor.matmul(warm_ps3[:], warm_sb[:, 0:128], warm_sb[:],
                                 start=(wi == 0), stop=(wi == 7))
            blocks[-1].finish()
            for ct in range(IT):
                op_final(ct)

    nc.compile()
    return nc


def prep_host(inputs, dt_mm):
    """Fold BN + scale + v-bias into effective weights (fp32 numpy)."""
    x = np.asarray(inputs["x"], dtype=np.float32)
    g = np.asarray(inputs["bn_gamma"], dtype=np.float32)
    be = np.asarray(inputs["bn_beta"], dtype=np.float32)
    mu = np.asarray(inputs["bn_mean"], dtype=np.float32)
    var = np.asarray(inputs["bn_var"], dtype=np.float32)
    wq = np.asarray(inputs["wq"], dtype=np.float32)
    bq = np.asarray(inputs["bq"], dtype=np.float32)
    wk = np.asarray(inputs["wk"], dtype=np.float32)
    bk = np.asarray(inputs["bk"], dtype=np.float32)
    wv = np.asarray(inputs["wv"], dtype=np.float32)
    bv = np.asarray(inputs["bv"], dtype=np.float32)
    wo = np.asarray(inputs["wo"], dtype=np.float32)
    bo = np.asarray(inputs["bo"], dtype=np.float32)

    a = g / np.sqrt(var + EPS)          # [C]
    bvec = be - mu * a                  # [C]

    wq_eff = wq * a[None, :] * SCALE
    bq_eff = (bq + wq @ bvec) * SCALE
    wk_eff = wk * a[None, :]
    bk_eff = bk + wk @ bvec
    wv_eff = wv * a[None, :]
    bv_eff = bv + wv @ bvec
    bo_eff = bo + wo @ bv_eff           # v bias rides through softmax (sums to 1)

    bias_pack = np.concatenate(
        [bq_eff.reshape(IT, 128).T, bk_eff.reshape(IT, 128).T,
         bo_eff.reshape(IT, 128).T], axis=1
    ).astype(np.float32)

    np_dt = np.float16 if mybir.dt.size(dt_mm) == 2 else np.float32

    def dev_layout(a_):
        # [C_or_I, N] -> [128, KC, N]: partition p holds rows {k*128+p}
        return np.ascontiguousarray(
            a_.reshape(KC, 128, a_.shape[1]).transpose(1, 0, 2).astype(np_dt))

    def dev_layout_hp(a_):
        # [C, I] -> [128, IT, KC, 128]: head-pair-major so the first
        # head-pair's weights are one contiguous DMA
        return np.ascontiguousarray(
            a_.reshape(KC, 128, IT, 128).transpose(1, 2, 0, 3).astype(np_dt))

    wq_l = dev_layout_hp(wq_eff.T)
    wk_l = dev_layout_hp(wk_eff.T)
    wv_l = dev_layout(wv_eff.T)
    wo_l = dev_layout(wo.T)
    per_core = []
    for b in range(B):
        per_core.append({
            "x": dev_layout(x[b, :, :, 0]),
            "wqT": wq_l,
            "wkT": wk_l,
            "wvT": wv_l,
            "woT": wo_l,
            "bias_pack": np.ascontiguousarray(bias_pack),
        })
    return per_core


def _get_nc(dt_mm):
    key = str(dt_mm)
    if key not in _CACHE:
        _CACHE[key] = build_bass(dt_mm)
    return _CACHE[key]


def kernel(**inputs):
    nc = _get_nc(DT_MM)
    in_maps = prep_host(inputs, DT_MM)
    res = run_bass_kernel_spmd(nc, in_maps, list(range(N_CORES)))
    y = np.stack([res.results[c]["y"] for c in range(N_CORES)], axis=0)
    return y[..., None].astype(np.float32)


def run_traced(**inputs):
    """Like kernel() but with NTFF profiling; returns (y, results, tmpdir)."""
    nc = _get_nc(DT_MM)
    in_maps = prep_host(inputs, DT_MM)
    import tempfile
    tmpdir = tempfile.mkdtemp(prefix="mha_trace_")
    res = run_bass_kernel_spmd(
        nc, in_maps, list(range(N_CORES)), trace=True, tmpdir=tmpdir
    )
    y = np.stack([res.results[c]["y"] for c in range(N_CORES)], axis=0)
    return y[..., None].astype(np.float32), res, tmpdir


# revision 28
# speedup vs baseline: 1.0594x; 1.0062x over previous
"""Trainium2 Bass kernel for BatchNorm2d + 8-head self-attention block.

Reference (per batch element b, all fp32):
    xn = BN_eval(x[b]); t = xn.T
    q/k/v = t @ W.T + b            # [S, 512], 8 heads x 64
    attn  = softmax(q k^T / 8)     # per head
    y[b]  = ((attn v) @ wo.T + bo).T

Sharding: pure data parallel — one batch element per NeuronCore, weights
replicated, no collectives.

Device design (per core), fully in the "transposed" domain (no large
transposes anywhere):
  - BN folded into QKV weights/biases on host; 1/8 scale folded into wq/bq;
    v bias folded into bo (softmax rows sum to 1).
  - Q^T,K^T [I,S] = wT.T @ x      (x arrives [C,S] — natural rhs)
  - V [S,I]       = x_chunk.T @ wvT, stored interleaved per head with a
    ones column ([128, 8*65]) so the PV matmul (M=65) also produces the
    softmax denominators for free.
  - scores^T per head [t,s]; head pairs row-packed via tile_position
    (0,0)/(64,0), K=64 each — the two row-group matmuls co-stream, so a
    scores pair costs one N=512 slot (~226ns). exp on ScalarE over both
    heads in one call (no max subtraction — scores are in [-3, 3]).
  - o^T accumulates over 8 t-chunks (K=128); normalize = stage the
    denominator row to SBUF (DVE's iterative-divide op must not read
    PSUM directly — doing so NaNs on hardware), approx-reciprocal,
    gpsimd partition-broadcast, DVE multiply; y^T = woT.T @ o^T + bo.

The wall-clock is paced by the ScalarE exp stream (64 ACTIVATEs x ~1.11us
on [128,1024]); the schedule keeps that stream dense:
  - ACT exp-table prefetched at t~0 with a dummy activation (saves the
    ~2.7us table-load off the first real exp); ~4.7us of dummy matmuls
    keep the PE's HAM clock-gate at 8/8 through the initial DMA wait.
  - inputs ride BOTH HWDGE rings concurrently (x + wo on sync, weights
    on the then-idle scalar ring), each ring a need-ordered priority
    chain; wq/wk live in head-pair-major DRAM layout so head-pair 0 is
    one contiguous early transfer. First exp fires ~15us in, not ~24us.
  - flat 64-chunk emission: scores pairs always lead; PV pairs are
    bunched into chunks 4-7 of each block (after the previous block's
    normalize has released the two po PSUM banks), and the next block's
    scores(0) is emitted between pv(6) and pv(7) so the exp stream
    crosses block boundaries with nothing in front of it. Projection /
    output fillers pop at chunks 1-3 from their own 2-bank PSUM pool,
    each pinned behind the current scores pair with a priority edge
    (the list-scheduler's optimistic DMA model would otherwise hoist
    DMA-gated fillers ahead of ready scores). The po*rbc normalize muls
    are deferred into the next block so the DVE FIFO is never
    head-blocked on the gpsimd broadcasts.
  - the denominator staging of the last block runs on the then-idle
    ScalarE, and dummy matmuls keep the PE warm through the final
    normalize so the last output projections run at full clock.

Matmul dtype is fp16: 2-byte weights keep LDWEIGHTS in the PE's
background buffer (hidden behind the previous matmul) where 4-byte fp32r
weights serialize ~150ns per matmul; fp16's 10-bit mantissa keeps the end
to-end error ~1e-3 of scale (all activations are within [-20, 20]).
"""

import numpy as np

import concourse.bass as bass
import concourse.tile as tile
from concourse import bacc, mybir
from concourse.bass_utils import run_bass_kernel_spmd
from concourse.tile import add_dep_helper

B, C, S = 8, 512, 1024
H, DH, INNER = 8, 64, 512
EPS = 1e-5
SCALE = DH ** (-0.5)
N_CORES = 8
F32 = mybir.dt.float32
F16 = mybir.dt.float16

DT_MM = F16

_CACHE: dict = {}

KC = C // 128      # 4 contraction chunks over channels
IT = INNER // 128  # 4 tiles over inner dim (also head-pair index)
ST = S // 128      # 8 t-chunks
NSLAB = S // 512   # 2 s-slabs


def build_bass(dt_mm):
    assert mybir.dt.size(dt_mm) == 2, "fp16/bf16 only"
    nc = bacc.Bacc("TRN2", target_bir_lowering=False, debug=False,
                   num_devices=N_CORES)

    # inputs arrive pre-arranged on the host into the SBUF layout
    # [partition, (head-pair,) k-chunk, free] so every DMA is contiguous
    # per partition AND the first head-pair of wq/wk is one transfer
    x_d = nc.dram_tensor("x", [128, KC, S], dt_mm, kind="ExternalInput")
    wqT_d = nc.dram_tensor("wqT", [128, IT, KC, 128], dt_mm,
                           kind="ExternalInput")
    wkT_d = nc.dram_tensor("wkT", [128, IT, KC, 128], dt_mm,
                           kind="ExternalInput")
    wvT_d = nc.dram_tensor("wvT", [128, KC, 512], dt_mm, kind="ExternalInput")
    woT_d = nc.dram_tensor("woT", [128, KC, 512], dt_mm, kind="ExternalInput")
    # bq | bk | bo packed on host as [128, 12] (col t+0/4/8 = vec[t*128+p])
    bias_d = nc.dram_tensor("bias_pack", [128, 3 * IT], F32, kind="ExternalInput")
    y_d = nc.dram_tensor("y", [C, S], F32, kind="ExternalOutput")

    with tile.TileContext(nc) as tc:
        with (
            tc.tile_pool(name="persist", bufs=1) as persist,
            tc.tile_pool(name="stage", bufs=2) as stage,
            tc.tile_pool(name="out", bufs=4) as outp,
            tc.tile_pool(name="norm", bufs=2) as normp,
            # one shared 4-slot pool for every 1-bank accumulator (projection
            # groups AND the two attention po accumulators): a fresh bank is
            # always available at head-pair transitions, so the strict-FIFO
            # PE queue never stalls behind the normalize chain.
            tc.tile_pool(name="psA", bufs=2, space="PSUM") as psA,
            tc.tile_pool(name="psP", bufs=2, space="PSUM") as psP,
            tc.tile_pool(name="psS", bufs=2, space="PSUM") as psS,
        ):
            xr = persist.tile([128, KC, S], dt_mm, tag="xr", name="xr")
            wqr = persist.tile([128, IT, KC, 128], dt_mm, tag="wqr", name="wqr")
            wkr = persist.tile([128, IT, KC, 128], dt_mm, tag="wkr", name="wkr")
            wvr = persist.tile([128, KC, 512], dt_mm, tag="wvr", name="wvr")
            wor = persist.tile([128, KC, 512], dt_mm, tag="wor", name="wor")

            # tiny bias pack rides the gpsimd/SWDGE queue in parallel with
            # the main chain; issue first so the Q7 starts immediately
            bias_sb = persist.tile([128, 3 * IT], F32, tag="bias")
            nc.gpsimd.dma_start(bias_sb[:], bias_d[:])
            bq_sb = bias_sb[:, 0:IT]
            bk_sb = bias_sb[:, IT:2 * IT]
            bo_sb = bias_sb[:, 2 * IT:3 * IT]

            # ---- main loads on BOTH HWDGE rings (sync + scalar), each a
            # need-ordered priority chain: a single ring streams ~140GB/s,
            # so x rides sync while the weights ride the scalar ring
            # concurrently (the scalar queue is idle until the first exp).
            dmas_sp = [
                nc.sync.dma_start(xr[:, :, 0:512], x_d[:, :, 0:512]),
                nc.sync.dma_start(xr[:, :, 512:1024], x_d[:, :, 512:1024]),
                nc.sync.dma_start(wor[:], woT_d[:]),
            ]
            dmas_act = [
                nc.scalar.dma_start(wqr[:, 0:1], wqT_d[:, 0:1]),
                nc.scalar.dma_start(wkr[:, 0:1], wkT_d[:, 0:1]),
                nc.scalar.dma_start(wvr[:], wvT_d[:]),
                nc.scalar.dma_start(wqr[:, 1:IT], wqT_d[:, 1:IT]),
                nc.scalar.dma_start(wkr[:, 1:IT], wkT_d[:, 1:IT]),
            ]
            for chain in (dmas_sp, dmas_act):
                for a, b in zip(chain[1:], chain):
                    add_dep_helper(a.ins, b.ins, sync=False,
                                   reason="dma priority")

            ones_sb = persist.tile([128, H], F32, tag="ones")
            nc.vector.memset(ones_sb[:], 1.0)

            # ---- PE warmup + ACT table prefetch during the initial DMA
            # wait: ~2.6us of dummy matmuls keeps the HAM clock-gate fed
            # until real work arrives; the dummy exp pulls the ~2.7us
            # exp_and_others table load off the first real activation ----
            warm_sb = stage.tile([128, 256], dt_mm, tag="warm", bufs=1)
            nc.vector.memset(warm_sb[:], 0.0)
            dummy_act = stage.tile([1, 8], F32, tag="dummy", bufs=1)
            nc.scalar.activation(dummy_act[:], warm_sb[0:1, 0:8],
                                 mybir.ActivationFunctionType.Exp)
            warm_ps = psA.tile([128, 256], F32, tag="acc", name="warm_ps")
            NWARM = 22
            for wi in range(NWARM):
                nc.tensor.matmul(warm_ps[:], warm_sb[:, 0:128], warm_sb[:],
                                 start=(wi == 0), stop=(wi == NWARM - 1))

            # ---- persistent per-slab outputs ----
            qT = [[persist.tile([128, 512], dt_mm, tag=f"qT{i}{s}",
                                name=f"qT{i}{s}") for s in range(NSLAB)]
                  for i in range(IT)]
            kT = [[persist.tile([128, 512], dt_mm, tag=f"kT{i}{s}",
                                name=f"kT{i}{s}") for s in range(NSLAB)]
                  for i in range(IT)]
            oT = [[persist.tile([128, 512], dt_mm, tag=f"oT{i}{s}",
                                name=f"oT{i}{s}") for s in range(NSLAB)]
                  for i in range(IT)]
            v_sb = [persist.tile([128, H * 65], dt_mm, tag=f"v{t}",
                                 name=f"v{t}") for t in range(ST)]
            # 8 persistent exp tiles round-robin (instead of a pool): the
            # WAR on slot reuse then collapses into the same PE semaphore
            # the scores wait already uses, saving an EVENT_SEMAPHORE on
            # the ScalarE queue per chunk
            et_sb = [persist.tile([128, 1024], dt_mm, tag=f"et{j}",
                                  name=f"et{j}") for j in range(8)]
            et_ctr = [0]

            def group_thunks(n_mm, emit_mm, evac):
                """n_mm single-matmul thunks accumulating into one psA bank;
                the first allocates the bank, the last appends the evacuation.
                Each thunk returns the matmul instruction it emitted so the
                scheduler can pin it behind the current scores pair."""
                box = []

                def mk(i):
                    def t():
                        if i == 0:
                            box.append(psA.tile([128, 512], F32,
                                                tag="acc", name="acc"))
                        ins = emit_mm(box[0], i)
                        if i == n_mm - 1:
                            evac(box[0])
                        return ins
                    return t

                return [mk(i) for i in range(n_mm)]

            def qk_thunks(w, bias, dst, hp, sl, evac_eng="vector"):
                def emit_mm(ps, kc):
                    return nc.tensor.matmul(
                        ps[:],
                        w[:, hp, kc, :],
                        xr[:, kc, sl * 512:(sl + 1) * 512],
                        start=(kc == 0), stop=(kc == KC - 1),
                    )

                def evac(ps):
                    if evac_eng == "scalar":
                        # ScalarE is idle before the first exp; Copy is in
                        # every ACT table set and bias is a per-partition
                        # pointer, so this runs the bias-add off the DVE
                        nc.scalar.activation(
                            dst[hp][sl][:], ps[:],
                            mybir.ActivationFunctionType.Copy,
                            bias=bias[:, hp:hp + 1],
                        )
                    else:
                        nc.vector.tensor_scalar_add(
                            dst[hp][sl][:], ps[:], bias[:, hp:hp + 1]
                        )

                return group_thunks(KC, emit_mm, evac)

            def v_thunks(tc_):
                def emit_mm(ps, kc):
                    return nc.tensor.matmul(
                        ps[:],
                        xr[:, kc, tc_ * 128:(tc_ + 1) * 128],
                        wvr[:, kc, :],
                        start=(kc == 0), stop=(kc == KC - 1),
                    )

                def evac(ps):
                    vv = v_sb[tc_][:].rearrange("p (h m) -> p h m", h=H)
                    nc.vector.tensor_copy(
                        vv[:, :, 0:64], ps[:].rearrange("p (h m) -> p h m", h=H)
                    )
                    nc.vector.tensor_copy(vv[:, :, 64:65], ones_sb[:, :, None])

                return group_thunks(KC, emit_mm, evac)

            def run(thunks):
                for t in thunks:
                    t()

            class Blk:
                """One (slab, head-pair) attention block's emission pieces.

                Per-chunk emission order in the flat scheduler below:
                scores -> [PV pairs] -> fillers. PVs are BUNCHED into
                chunks 4-7 (two pairs per chunk, S+4PV = 1075ns < the
                1113ns exp), so this block's po PSUM banks are first
                written ~5.5us in — after the previous block's normalize
                has released its pair. The next block's scores(0) is
                emitted BETWEEN pv(6) and pv(7) so the exp stream crosses
                block boundaries with no PE work in front of it.
                """

                def __init__(self, sl, hp):
                    self.sl, self.hp = sl, hp
                    self.ets = []
                    self.po = None
                    self.rbc = None
                    self.last_mm = None

                def scores(self, tc_):
                    ksl, kcol = tc_ // 4, (tc_ % 4) * 128
                    pss = psS.tile([128, 1024], F32, tag="psS", name="psS")
                    nc.tensor.matmul(
                        pss[:, 0:512],
                        kT[self.hp][ksl][0:64, kcol:kcol + 128],
                        qT[self.hp][self.sl][0:64, :],
                        start=True, stop=True, tile_position=(0, 0),
                    )
                    self.last_mm = nc.tensor.matmul(
                        pss[:, 512:1024],
                        kT[self.hp][ksl][64:128, kcol:kcol + 128],
                        qT[self.hp][self.sl][64:128, :],
                        start=True, stop=True, tile_position=(64, 0),
                    )
                    et = et_sb[et_ctr[0] % 8]
                    et_ctr[0] += 1
                    nc.scalar.activation(
                        et[:], pss[:], mybir.ActivationFunctionType.Exp
                    )
                    self.ets.append(et)

                def pv(self, tc_):
                    if tc_ == 0:
                        self.po = (
                            psP.tile([65, 512], F32, tag="po", name="po0"),
                            psP.tile([65, 512], F32, tag="po", name="po1"),
                        )
                    for half in (0, 1):
                        h = 2 * self.hp + half
                        nc.tensor.matmul(
                            self.po[half][:],
                            v_sb[tc_][:, h * 65:(h + 1) * 65],
                            self.ets[tc_][:, half * 512:(half + 1) * 512],
                            start=(tc_ == 0), stop=(tc_ == ST - 1),
                        )

                def norm_front(self, last=False):
                    # stage the denominator rows to SBUF (DVE's iterative-
                    # divide op must not read PSUM directly; in the tail
                    # the idle ScalarE does the staging), then reciprocal +
                    # gpsimd partition-broadcast. The po*rbc muls are
                    # deferred into the NEXT block's emission (finish) so
                    # the DVE FIFO isn't head-blocked waiting on the
                    # broadcasts while later work is ready to run.
                    rrow = []
                    for half in (0, 1):
                        po = self.po[half]
                        dr = normp.tile([1, 512], F32, tag=f"drow{half}",
                                        name="drow")
                        if last:
                            nc.scalar.copy(dr[:], po[64:65, :])
                        else:
                            nc.vector.tensor_copy(dr[:], po[64:65, :])
                        rr = normp.tile([1, 512], F32, tag=f"rrow{half}",
                                        name="rrow")
                        nc.vector.reciprocal_approx_fast(rr[:], dr[:])
                        rrow.append(rr)
                    self.rbc = []
                    for half in (0, 1):
                        rb = normp.tile([64, 512], F32, tag=f"rbc{half}",
                                        name="rbc")
                        nc.gpsimd.partition_broadcast(rb[:], rrow[half][:])
                        self.rbc.append(rb)

                def finish(self):
                    for half in (0, 1):
                        nc.vector.tensor_mul(
                            oT[self.hp][self.sl][half * 64:(half + 1) * 64, :],
                            self.po[half][0:64, :],
                            self.rbc[half][:],
                        )

            def op_thunks(sl, ct):
                def emit_mm(ps, ic):
                    return nc.tensor.matmul(
                        ps[:],
                        wor[:, ic, ct * 128:(ct + 1) * 128],
                        oT[ic][sl][:],
                        start=(ic == 0), stop=(ic == IT - 1),
                    )

                def evac(ps):
                    ysb = outp.tile([128, 512], F32, tag="ysb", name="ysb")
                    nc.vector.tensor_scalar_add(ysb[:], ps[:],
                                                bo_sb[:, ct:ct + 1])
                    nc.sync.dma_start(
                        y_d[ct * 128:(ct + 1) * 128,
                            sl * 512:(sl + 1) * 512],
                        ysb[:],
                    )

                return group_thunks(IT, emit_mm, evac)

            y_part = [persist.tile([128, 512], F32, tag=f"yp{ct}",
                                   name=f"yp{ct}") for ct in range(IT)]

            def op_partial_thunks(ct):
                # ic 0..2 of the sl=1 projection, banked into SBUF (+bias)
                def emit_mm(ps, ic):
                    return nc.tensor.matmul(
                        ps[:],
                        wor[:, ic, ct * 128:(ct + 1) * 128],
                        oT[ic][1][:],
                        start=(ic == 0), stop=(ic == IT - 2),
                    )

                def evac(ps):
                    nc.vector.tensor_scalar_add(y_part[ct][:], ps[:],
                                                bo_sb[:, ct:ct + 1])

                return group_thunks(IT - 1, emit_mm, evac)

            def op_final(ct):
                ps = psA.tile([128, 512], F32, tag="acc", name="acc")
                nc.tensor.matmul(
                    ps[:],
                    wor[:, IT - 1, ct * 128:(ct + 1) * 128],
                    oT[IT - 1][1][:],
                    start=True, stop=True,
                )
                ysb = outp.tile([128, 512], F32, tag="ysb", name="ysb")
                nc.vector.tensor_add(ysb[:], y_part[ct][:], ps[:])
                nc.sync.dma_start(
                    y_d[ct * 128:(ct + 1) * 128, 512:1024], ysb[:],
                )

            # ---- emission order = static scheduler priority. Fillers are
            # single-matmul thunks so the exp-paced attention loop is never
            # blocked by a multi-matmul projection block sitting ahead of
            # the next scores in the PE's strict-FIFO queue.
            # Naming: Q(hp,sl)/K(hp,sl) 4-mm groups; V(t) 4-mm groups.
            # Deps: att(0,hp) reads q/k (hp,0) at chunk 0 and k (hp,1) at
            # chunk 4; v_sb[t] must land before chunk t's PV; oT[*][0] is
            # ready ~1.5us into block 4; oT[i][1] after block 4+i's
            # normalize. ----
            def Q(hp, sl):
                return qk_thunks(wqr, bq_sb, qT, hp, sl)

            def K(hp, sl):
                return qk_thunks(wkr, bk_sb, kT, hp, sl)

            # pre-phase (overlaps the DMA chains): first scores chunk needs
            # only Q(0,0)+K(0,0). Block 0 carries all eight V groups plus
            # the projections the next block reads; later blocks pop their
            # successors' q/k/op groups at chunks 0-2, by which point the
            # previous block's normalize has released the shared filler
            # PSUM banks. K(hp,1) is read by a block's own scores chunk 4,
            # so it pops at chunk 3 at the latest (or a block early).
            run(Q(0, 0))
            run(K(0, 0))
            # keep the PE's HAM clock-gate warm through the evac wait
            # before the first scores pair
            for wi in range(6):
                warm_ps2 = psA.tile([128, 256], F32, tag="acc",
                                    name="warm2") if wi == 0 else warm_ps2
                nc.tensor.matmul(warm_ps2[:], warm_sb[:, 0:128], warm_sb[:],
                                 start=(wi == 0), stop=(wi == 5))
            opc = [op_thunks(0, ct) for ct in range(IT)]
            # op1p thunk order staggers each 3-matmul group so its third
            # matmul (which reads oT[2][1], ready only ~2.5us into block
            # 7 once block 6's deferred normalize muls land) pops two
            # chunks after the group opens, and group lifetimes alternate
            # between the two filler PSUM banks with no overlap stalls:
            # A01|B01|A2|C01|B2|D01|C2|D2 over chunks 1..7 + drain.
            opg = [op_partial_thunks(ct) for ct in range(IT)]
            gA, gB, gC, gD = opg
            op1p = (gA[0:2] + gB[0:2] + gA[2:3] + gC[0:2]
                    + gB[2:3] + gD[0:2] + gC[2:3] + gD[2:3])
            fillers_tbl = [
                (v_thunks(0) + v_thunks(1) + v_thunks(2) + v_thunks(3)
                 + v_thunks(4) + v_thunks(5) + K(0, 1) + v_thunks(6)
                 + v_thunks(7) + Q(1, 0) + K(1, 0)),
                K(1, 1) + K(2, 1) + Q(2, 0) + K(2, 0),
                K(3, 1) + Q(3, 0) + K(3, 0),
                Q(0, 1) + Q(1, 1),
                Q(2, 1) + Q(3, 1),
                opc[0] + opc[1],
                opc[2] + opc[3],
                op1p,
            ]
            # pops start at chunk 1 so nothing sits between the boundary
            # scores pair and the next chunk's pair; blocks 6/7 carry the
            # output-projection work so the PE never runs so far ahead of
            # the exp stream that the psS reuse turns into a semaphore
            # ping-pong between the two engines.
            pops_tbl = [
                (8, 8, 8, 8, 8, 4, 0, 0),
                (0, 4, 4, 8, 0, 0, 0, 0),
                (0, 4, 4, 4, 0, 0, 0, 0),
                (0, 4, 4, 0, 0, 0, 0, 0),
                (0, 4, 4, 0, 0, 0, 0, 0),
                (0, 4, 4, 0, 0, 0, 0, 0),
                (0, 4, 4, 0, 0, 0, 0, 0),
                (0, 2, 2, 1, 2, 1, 2, 0),
            ]
            blocks = [Blk(sl, hp) for sl in range(NSLAB) for hp in range(IT)]

            def pop_filler(fillers, B):
                # every filler matmul gets a priority edge behind the most
                # recent scores pair: the Tile list-scheduler's DMA cost
                # model is optimistic, and without the edge it hoists
                # DMA-gated fillers ahead of ready scores in the PE FIFO,
                # stalling the exp stream behind the real (late) DMA.
                ins = fillers.pop(0)()
                if ins is not None and B.last_mm is not None:
                    add_dep_helper(ins.ins, B.last_mm.ins, sync=False,
                                   reason="filler after scores")

            for b, B in enumerate(blocks):
                fillers = list(fillers_tbl[b])
                pops = pops_tbl[b]
                pv_start = 4
                pv_done = 0
                for tc_ in range(ST):
                    if tc_ > 0 or b == 0:
                        B.scores(tc_)
                    if tc_ >= pv_start:
                        want = min(2 * (tc_ - pv_start + 1), ST)
                        while pv_done < want and pv_done <= tc_:
                            if pv_done == ST - 1 and b + 1 < len(blocks):
                                blocks[b + 1].scores(0)
                            B.pv(pv_done)
                            pv_done += 1
                    if tc_ == 1 and b > 0:
                        # before the pops: b7's op1p fillers read oT[2][1],
                        # which this finish writes — emission order is
                        # dependency order
                        blocks[b - 1].finish()
                    for _ in range(pops[tc_]):
                        if fillers:
                            pop_filler(fillers, B)
                B.norm_front(last=(b == len(blocks) - 1))
                while fillers:
                    pop_filler(fillers, B)
            # keep the PE warm through the final normalize chain so the
            # op_final matmuls run at full clock
            warm_ps3 = psA.tile([128, 256], F32, tag="acc", name="warm3")
            for wi in range(8):
                nc.tensor.matmul(warm_ps3[:], warm_sb[:, 0:128], warm_sb[:],
                                 start=(wi == 0), stop=(wi == 7))
            blocks[-1].finish()
            for ct in range(IT):
                op_final(ct)

    nc.compile()
    return nc


def prep_host(inputs, dt_mm):
    """Fold BN + scale + v-bias into effective weights (fp32 numpy)."""
    x = np.asarray(inputs["x"], dtype=np.float32)
    g = np.asarray(inputs["bn_gamma"], dtype=np.float32)
    be = np.asarray(inputs["bn_beta"], dtype=np.float32)
    mu = np.asarray(inputs["bn_mean"], dtype=np.float32)
    var = np.asarray(inputs["bn_var"], dtype=np.float32)
    wq = np.asarray(inputs["wq"], dtype=np.float32)
    bq = np.asarray(inputs["bq"], dtype=np.float32)
    wk = np.asarray(inputs["wk"], dtype=np.float32)
    bk = np.asarray(inputs["bk"], dtype=np.float32)
    wv = np.asarray(inputs["wv"], dtype=np.float32)
    bv = np.asarray(inputs["bv"], dtype=np.float32)
    wo = np.asarray(inputs["wo"], dtype=np.float32)
    bo = np.asarray(inputs["bo"], dtype=np.float32)

    a = g / np.sqrt(var + EPS)          # [C]
    bvec = be - mu * a                  # [C]

    wq_eff = wq * a[None, :] * SCALE
    bq_eff = (bq + wq @ bvec) * SCALE
    wk_eff = wk * a[None, :]
    bk_eff = bk + wk @ bvec
    wv_eff = wv * a[None, :]
    bv_eff = bv + wv @ bvec
    bo_eff = bo + wo @ bv_eff           # v bias rides through softmax (sums to 1)

    bias_pack = np.concatenate(
        [bq_eff.reshape(IT, 128).T, bk_eff.reshape(IT, 128).T,
         bo_eff.reshape(IT, 128).T], axis=1
    ).astype(np.float32)

    np_dt = np.float16 if mybir.dt.size(dt_mm) == 2 else np.float32

    def dev_layout(a_):
        # [C_or_I, N] -> [128, KC, N]: partition p holds rows {k*128+p}
        return np.ascontiguousarray(
            a_.reshape(KC, 128, a_.shape[1]).transpose(1, 0, 2).astype(np_dt))

    def dev_layout_hp(a_):
        # [C, I] -> [128, IT, KC, 128]: head-pair-major so the first
        # head-pair's weights are one contiguous DMA
        return np.ascontiguousarray(
            a_.reshape(KC, 128, IT, 128).transpose(1, 2, 0, 3).astype(np_dt))

    wq_l = dev_layout_hp(wq_eff.T)
    wk_l = dev_layout_hp(wk_eff.T)
    wv_l = dev_layout(wv_eff.T)
    wo_l = dev_layout(wo.T)
    per_core = []
    for b in range(B):
        per_core.append({
            "x": dev_layout(x[b, :, :, 0]),
            "wqT": wq_l,
            "wkT": wk_l,
            "wvT": wv_l,
            "woT": wo_l,
            "bias_pack": np.ascontiguousarray(bias_pack),
        })
    return per_core


def _get_nc(dt_mm):
    key = str(dt_mm)
    if key not in _CACHE:
        _CACHE[key] = build_bass(dt_mm)
    return _CACHE[key]


def kernel(**inputs):
    nc = _get_nc(DT_MM)
    in_maps = prep_host(inputs, DT_MM)
    res = run_bass_kernel_spmd(nc, in_maps, list(range(N_CORES)))
    y = np.stack([res.results[c]["y"] for c in range(N_CORES)], axis=0)
    return y[..., None].astype(np.float32)


def run_traced(**inputs):
    """Like kernel() but with NTFF profiling; returns (y, results, tmpdir)."""
    nc = _get_nc(DT_MM)
    in_maps = prep_host(inputs, DT_MM)
    import tempfile
    tmpdir = tempfile.mkdtemp(prefix="mha_trace_")
    res = run_bass_kernel_spmd(
        nc, in_maps, list(range(N_CORES)), trace=True, tmpdir=tmpdir
    )
    y = np.stack([res.results[c]["y"] for c in range(N_CORES)], axis=0)
    return y[..., None].astype(np.float32), res, tmpdir
